# revision 6
# baseline (speedup 1.0000x reference)
"""Trainium2 Bass kernel for nn_Attention_80779744903968.

Reference computation (B=32, T=512, S=1024, H=1024):
    z      = q @ W_in.T                  [B,T,H]
    scores = z @ enc_b.T                 [B,T,S]   (enc input is [S,B,H])
    p      = softmax(scores, axis=-1)    (the scores==0 -> -inf fill is a
                                          numerical no-op: row maxes are ~120,
                                          exp(0-max) == 0 in fp32)
    c      = p @ enc_b                   [B,T,H]
    out    = tanh([c, q] @ W_out.T + b)  [B,T,H]

Sharding: data-parallel over B across 8 cores (4 batches per core).
W_in / W_out replicated.

Precision strategy (PE matmuls):
  - z and scores need near-fp32 logits: the softmax is near-one-hot
    (logit std ~37) with near-tied rows (min top-2 gap ~2e-4), so every
    one of the four correction products matters for the absmax.
  - Both are computed as an fp16 hi/lo split: x*y ~= xh*yh (fp16 main
    pass, fp32 PSUM accumulation) + (xh*yl + xl*yh) correction passes in
    fp8(e4m3) with perf_mode=DoubleRow at 2 contraction-tiles per
    matmul; correction operands are pre-scaled by powers of 2 so both
    corr products share one PSUM scale.
  - downstream (p, enc, c, q, W_out) runs in plain fp16: p is in [0,1]
    and c/out magnitudes are O(1), so fp16's 2^-11 relative error is
    plenty.

Schedule notes (perf):
  - z and scores phases run quad-grouped: 32 fp16 main matmuls, then 32
    fp8-DR correction matmuls (pass-major).  fp16<->fp8 PE mode switches
    cost ~0.4us each, so fewer/longer runs win.
  - hi/lo merge is a single scalar_tensor_tensor reading BOTH psum banks
    (main + corr), freeing them in one DVE op.
  - softmax: exp runs on the Scalar engine with accum_out producing the
    row sum for free; p is NOT normalized -- the 1/sum scale is applied
    at the out-projection eviction (out rows have t on partitions, so
    rsum is a per-partition scalar there).  The out projection therefore
    accumulates its c-part and q-part in separate psum banks.
  - p is transposed by the DMA xbar (SBUF->SBUF, fp16) instead of the
    PE; the c matmul consumes the transposed tile directly.

All input transposes (q -> [H,T], enc -> [H,S] per batch) are done on
the host so every device-side input DMA is a contiguous natural-layout
load.  Output is written fp16 and cast to fp32 on the host.
"""
import os
import sys

import numpy as np

sys.path.insert(0, "/opt/trn_rl_repo")

import ml_dtypes  # noqa: E402

import concourse.bass as bass  # noqa: E402
import concourse.tile as tile  # noqa: E402
from concourse import bacc, mybir  # noqa: E402
from concourse.bass_utils import run_bass_kernel_spmd  # noqa: E402
from concourse.masks import make_identity  # noqa: E402

B, T, S, H = 32, 512, 1024, 1024
NCORES = 8
BL = B // NCORES  # batches per core
HT = H // 128     # h/i/k tiles per 1024
TT = T // 128     # t tiles
ST = S // 128     # s tiles
F16 = mybir.dt.float16
F32 = mybir.dt.float32
F8 = mybir.dt.float8e4
DR = mybir.MatmulPerfMode.DoubleRow

TR = os.environ.get("KERNEL_TR", "dma")      # p-transpose: dma xbar | pe
NORM = os.environ.get("KERNEL_NORM", "defer")  # softmax norm: defer | now

# power-of-2 scales for fp8 correction operands (products must share scale)
SC_WH, SC_WL, SC_QH, SC_QL = 2.0**4, 2.0**15, 1.0, 2.0**11   # z corr: 2^15
SC_ZH, SC_ZL, SC_EH, SC_EL = 1.0, 2.0**12, 1.0, 2.0**12      # s corr: 2^12

_CACHE = {}


def _build(has_bias):
    nc = bacc.Bacc("TRN2", target_bir_lowering=False, debug=False,
                   num_devices=NCORES)

    def din(name, shape, dt=F16):
        return nc.dram_tensor(name, shape, dt, kind="ExternalInput").ap()

    qh_d = din("qh", [BL, H, T])
    eh_d = din("eh", [BL, H, S])
    en_d = din("en", [BL, S, H])
    wh_d = din("wh", [H, H])
    wo_d = din("wo", [2 * H, H])
    if has_bias:
        bias_d = din("bias", [128, H], F32)
    ql8_d = din("ql8", [BL, H, T], F8)
    el8_d = din("el8", [BL, H, S], F8)
    qh8_d = din("qh8", [BL, H, T], F8)
    eh8_d = din("eh8", [BL, H, S], F8)
    wh8_d = din("wh8", [H, H], F8)
    wl8_d = din("wl8", [H, H], F8)
    out_d = nc.dram_tensor("out", [BL, T, H], F16, kind="ExternalOutput").ap()

    with tile.TileContext(nc) as tc:
        with (
            tc.tile_pool(name="weights", bufs=1) as wp,
            tc.tile_pool(name="qin", bufs=2) as qp,
            tc.tile_pool(name="ein", bufs=1) as ep,
            tc.tile_pool(name="enin", bufs=1) as enp,
            tc.tile_pool(name="zbuf", bufs=1) as zp,
            tc.tile_pool(name="scratch", bufs=2) as scrp,
            tc.tile_pool(name="scores", bufs=2) as scp,
            tc.tile_pool(name="pbuf", bufs=3) as pp,
            tc.tile_pool(name="ptbuf", bufs=1) as ptp,
            tc.tile_pool(name="ctbuf", bufs=1) as ctp,
            tc.tile_pool(name="ostage", bufs=3) as op,
            tc.tile_pool(name="stats", bufs=8) as stp,
            tc.tile_pool(name="psmm", bufs=4, space="PSUM") as psmm,
            tc.tile_pool(name="psc", bufs=4, space="PSUM") as psc,
        ):
            # --- resident weights / constants ---
            # queue order matters at startup: chunk wh/qh0 by ht so the first
            # z matmul waits on ~384KB, not 3MB (DMA sems are per-transfer);
            # z corr pass 1 needs wl8+qh8 so those precede wh8/ql8.
            wh_t = wp.tile([128, HT, H], F16)
            wh_r = wh_d.rearrange("(ht p) i -> p ht i", p=128)
            qh_first = qp.tile([128, HT, T], F16, tag="qh")
            qh_r = qh_d[0].rearrange("(ht p) t -> p ht t", p=128)
            for ht in range(HT):
                nc.sync.dma_start(wh_t[:, ht, :], wh_r[:, ht, :])
                nc.sync.dma_start(qh_first[:, ht, :], qh_r[:, ht, :])
            wl8_t = wp.tile([128, HT, H], F8)
            nc.sync.dma_start(
                wl8_t[:], wl8_d.rearrange("(ht p) i -> p ht i", p=128))
            qh8_first = qp.tile([128, HT, T], F8, tag="qh8", bufs=1)
            nc.sync.dma_start(
                qh8_first[:], qh8_d[0].rearrange("(ht p) t -> p ht t", p=128))
            wh8_t = wp.tile([128, HT, H], F8)
            nc.sync.dma_start(
                wh8_t[:], wh8_d.rearrange("(ht p) i -> p ht i", p=128))
            ql8_first = qp.tile([128, HT, T], F8, tag="ql8", bufs=1)
            nc.sync.dma_start(
                ql8_first[:], ql8_d[0].rearrange("(ht p) t -> p ht t", p=128))
            if TR == "pe":
                ident = wp.tile([128, 128], F16)
                make_identity(nc, ident[:])
            wo_t = None
            bias_t = None

            for b in range(BL):
                if b == 0:
                    qh_t, qh8_t, ql8_t = qh_first, qh8_first, ql8_first
                else:
                    qh_t = qp.tile([128, HT, T], F16, tag="qh")
                    nc.sync.dma_start(
                        qh_t[:], qh_d[b].rearrange("(ht p) t -> p ht t", p=128))
                    qh8_t = qp.tile([128, HT, T], F8, tag="qh8", bufs=1)
                    nc.sync.dma_start(
                        qh8_t[:], qh8_d[b].rearrange("(ht p) t -> p ht t", p=128))
                    ql8_t = qp.tile([128, HT, T], F8, tag="ql8", bufs=1)
                    nc.sync.dma_start(
                        ql8_t[:], ql8_d[b].rearrange("(ht p) t -> p ht t", p=128))
                eh_t = ep.tile([128, HT, S], F16, tag="eh")
                nc.sync.dma_start(
                    eh_t[:], eh_d[b].rearrange("(it p) s -> p it s", p=128))
                eh8_t = ep.tile([128, HT, S], F8, tag="eh8")
                nc.sync.dma_start(
                    eh8_t[:], eh8_d[b].rearrange("(it p) s -> p it s", p=128))
                el8_t = ep.tile([128, HT, S], F8, tag="el8")
                nc.sync.dma_start(
                    el8_t[:], el8_d[b].rearrange("(it p) s -> p it s", p=128))
                en_t = enp.tile([128, ST, H], F16, tag="en")
                nc.sync.dma_start(
                    en_t[:], en_d[b].rearrange("(st p) k -> p st k", p=128))
                if wo_t is None:
                    # deferred: b0's z/scores inputs take queue priority
                    wo_t = wp.tile([128, 2 * HT, H], F16)
                    nc.sync.dma_start(
                        wo_t[:], wo_d.rearrange("(kt p) h -> p kt h", p=128))
                    if has_bias:
                        bias_t = wp.tile([128, H], F32)
                        nc.sync.dma_start(bias_t[:], bias_d)

                # --- zT = W_inT.T @ qT (hi/lo) -> zh (f16) + fp8 lo forms ---
                zh_t = zp.tile([128, HT, T], F16, tag="zh")
                zh8_t = zp.tile([128, HT, T], F8, tag="zh8")
                zl8_t = zp.tile([128, HT, T], F8, tag="zl8")

                for quad in range(HT // 4):
                    its = range(quad * 4, quad * 4 + 4)
                    zpss = {it: psmm.tile([128, T], F32, tag="mm",
                                          name=f"zps{it % 4}")
                            for it in its}
                    # main: 32 fp16 matmuls, ht-major so b0 overlaps the
                    # chunked wh/qh arrival
                    for ht in range(HT):
                        for it in its:
                            nc.tensor.matmul(
                                zpss[it][:],
                                wh_t[:, ht, it * 128:(it + 1) * 128],
                                qh_t[:, ht, :],
                                start=(ht == 0), stop=(ht == HT - 1))
                    # corr: 32 fp8-DR matmuls, pass-major (b0: pass 1 only
                    # needs wl8+qh8 which arrive before wh8/ql8)
                    zcorrs = {it: psc.tile([128, T], F32, tag="mmc",
                                           name=f"zcorr{it % 4}")
                             for it in its}
                    for pi, (lhs, rhs) in enumerate(
                            ((wl8_t, qh8_t), (wh8_t, ql8_t))):
                        for it in its:
                            for k in range(HT // 2):
                                nc.tensor.matmul(
                                    zcorrs[it][:],
                                    lhs[:, 2 * k:2 * k + 2,
                                        it * 128:(it + 1) * 128],
                                    rhs[:, 2 * k:2 * k + 2, :],
                                    start=(pi == 0 and k == 0),
                                    stop=(pi == 1 and k == HT // 2 - 1),
                                    perf_mode=DR)
                    # merge: comb = zps + zcorr * 2^-19 (DVE can read only one
                    # PSUM operand per op), then split into f16 + scaled f8 lo
                    for it in its:
                        comb = scrp.tile([128, T], F32, tag="comb")
                        nc.vector.tensor_copy(comb[:], zpss[it][:])
                        nc.vector.scalar_tensor_tensor(
                            out=comb[:], in0=zcorrs[it][:],
                            scalar=1.0 / (SC_WH * SC_QL), in1=comb[:],
                            op0=mybir.AluOpType.mult, op1=mybir.AluOpType.add)
                        nc.vector.tensor_copy(zh_t[:, it, :], comb[:])
                        zl_tmp = scrp.tile([128, T], F16, tag="zltmp")
                        nc.vector.tensor_sub(zl_tmp[:], comb[:],
                                             zh_t[:, it, :])
                        nc.vector.tensor_scalar_mul(
                            zl8_t[:, it, :], zl_tmp[:], SC_ZL)
                        nc.vector.tensor_copy(zh8_t[:, it, :], zh_t[:, it, :])

                # --- scores + softmax -> p (f16, unnormalized) ---
                # units u = (tt, sc): quad = 2 tt x 2 sc so softmax for a tt
                # can start right after its quad merges.
                p_tiles = []
                rsums = []
                sc_tiles = {}
                pt_t = ptp.tile([128, ST, T], F16, tag="pt")
                for quad in range(2):
                    units = [(quad * 2 + dt, sc) for dt in range(2)
                             for sc in range(2)]
                    spss = {}
                    for (tt, sc) in units:
                        if sc == 0:
                            sc_tiles[tt] = scp.tile([128, S], F32, tag="sc",
                                                    name=f"sc{tt % 2}")
                        sps = psmm.tile([128, 512], F32, tag="mm",
                                        name=f"sps{tt % 2}_{sc}")
                        spss[(tt, sc)] = sps
                        for it in range(HT):
                            nc.tensor.matmul(
                                sps[:],
                                zh_t[:, it, tt * 128:(tt + 1) * 128],
                                eh_t[:, it, sc * 512:(sc + 1) * 512],
                                start=(it == 0), stop=(it == HT - 1))
                    scorrs = {}
                    for (tt, sc) in units:
                        scorrs[(tt, sc)] = psc.tile([128, 512], F32, tag="mmc",
                                                    name=f"scorr{tt % 2}_{sc}")
                    for pi, (lhs, rhs) in enumerate(
                            ((zl8_t, eh8_t), (zh8_t, el8_t))):
                        for (tt, sc) in units:
                            for k in range(HT // 2):
                                nc.tensor.matmul(
                                    scorrs[(tt, sc)][:],
                                    lhs[:, 2 * k:2 * k + 2,
                                        tt * 128:(tt + 1) * 128],
                                    rhs[:, 2 * k:2 * k + 2,
                                        sc * 512:(sc + 1) * 512],
                                    start=(pi == 0 and k == 0),
                                    stop=(pi == 1 and k == HT // 2 - 1),
                                    perf_mode=DR)
                    for ui, (tt, sc) in enumerate(units):
                        chunk = sc_tiles[tt][:, sc * 512:(sc + 1) * 512]
                        nc.vector.tensor_copy(chunk, spss[(tt, sc)][:])
                        nc.vector.scalar_tensor_tensor(
                            out=chunk, in0=scorrs[(tt, sc)][:],
                            scalar=1.0 / (SC_ZL * SC_EH),
                            in1=chunk,
                            op0=mybir.AluOpType.mult,
                            op1=mybir.AluOpType.add)
                        if sc == 1:
                            # both chunks of tt merged: softmax over free dim
                            sc_t = sc_tiles[tt]
                            negmax = stp.tile([128, 1], F32, tag="nm")
                            nc.vector.reduce_max(
                                out=negmax[:], in_=sc_t[:],
                                axis=mybir.AxisListType.X, negate=True)
                            p_t = pp.tile([128, S], F16, tag="p")
                            ssum = stp.tile([128, 1], F32, tag="ss")
                            nc.scalar.activation(
                                out=p_t[:], in_=sc_t[:],
                                func=mybir.ActivationFunctionType.Exp,
                                bias=negmax[:], scale=1.0,
                                accum_out=ssum[:])
                            rsum = stp.tile([128, 1], F32, tag="rs", bufs=8)
                            nc.vector.reciprocal(rsum[:], ssum[:])
                            rsums.append(rsum)
                            if NORM == "now":
                                nc.vector.tensor_scalar_mul(
                                    p_t[:], p_t[:], rsum[:])
                            p_tiles.append(p_t)
                            # transpose p -> pT [s, t] via DMA xbar
                            if TR == "dma":
                                for st in range(ST):
                                    nc.sync.dma_start(
                                        pt_t[:, st, tt * 128:(tt + 1) * 128],
                                        p_t[:, st * 128:(st + 1) * 128],
                                        transpose=True)

                if TR == "pe":
                    for tt in range(TT):
                        for st in range(ST):
                            tps = psc.tile([128, 128], F16, tag="tr", bufs=2)
                            nc.tensor.transpose(
                                tps[:], p_tiles[tt][:, st * 128:(st + 1) * 128],
                                ident[:])
                            nc.vector.tensor_copy(
                                pt_t[:, st, tt * 128:(tt + 1) * 128], tps[:])

                # --- cT = enc_nat.T @ pT -> [k, t] f16 (unnormalized) ---
                ct_t = ctp.tile([128, HT, T], F16, tag="ct")
                for kt in range(HT):
                    cps = psmm.tile([128, T], F32, tag="mm")
                    for st in range(ST):
                        nc.tensor.matmul(
                            cps[:],
                            en_t[:, st, kt * 128:(kt + 1) * 128],
                            pt_t[:, st, :],
                            start=(st == 0), stop=(st == ST - 1))
                    nc.vector.tensor_copy(ct_t[:, kt, :], cps[:])

                # --- out = tanh(cT.T @ WcT * rsum + qT.T @ WqT + b) ---
                for tt in range(TT):
                    for hc in range(2):
                        # q-part first: gives tail cT evictions extra slack
                        oq = psc.tile([128, 512], F32, tag="mmc")
                        for ht in range(HT):
                            nc.tensor.matmul(
                                oq[:],
                                qh_t[:, ht, tt * 128:(tt + 1) * 128],
                                wo_t[:, HT + ht, hc * 512:(hc + 1) * 512],
                                start=(ht == 0), stop=(ht == HT - 1))
                        oc = psmm.tile([128, 512], F32, tag="mm")
                        for kt in range(HT):
                            nc.tensor.matmul(
                                oc[:],
                                ct_t[:, kt, tt * 128:(tt + 1) * 128],
                                wo_t[:, kt, hc * 512:(hc + 1) * 512],
                                start=(kt == 0), stop=(kt == HT - 1))
                        ost = op.tile([128, 512], F32 if has_bias else F16,
                                      tag="os")
                        if NORM == "defer":
                            ocn = op.tile([128, 512], F32, tag="ocn")
                            nc.vector.tensor_scalar_mul(
                                ocn[:], oc[:], rsums[tt][:])
                            nc.vector.tensor_add(ost[:], ocn[:], oq[:])
                        else:
                            nc.vector.tensor_add(ost[:], oc[:], oq[:])
                        if has_bias:
                            ost16 = op.tile([128, 512], F16, tag="os16")
                            nc.vector.tensor_add(
                                ost[:], ost[:],
                                bias_t[:, hc * 512:(hc + 1) * 512])
                            nc.scalar.activation(
                                out=ost16[:], in_=ost[:],
                                func=mybir.ActivationFunctionType.Tanh)
                            ost = ost16
                        else:
                            nc.scalar.activation(
                                out=ost[:], in_=ost[:],
                                func=mybir.ActivationFunctionType.Tanh)
                        nc.sync.dma_start(
                            out_d[b, tt * 128:(tt + 1) * 128,
                                  hc * 512:(hc + 1) * 512],
                            ost[:])

    nc.compile()
    return nc


def _get_nc(has_bias):
    key = ("nc", has_bias, TR, NORM)
    if key not in _CACHE:
        _CACHE[key] = _build(has_bias)
    return _CACHE[key]


def _split16(x):
    hi = x.astype(np.float16)
    lo = (x - hi.astype(np.float32)).astype(np.float32)
    return hi, lo


def _f8(x, scale):
    return (np.asarray(x, np.float32) * np.float32(scale)).astype(
        ml_dtypes.float8_e4m3)


def kernel(query, encoder_outputs, src_lengths, W_in, W_out, b_out):
    query = np.asarray(query, np.float32)
    enc = np.asarray(encoder_outputs, np.float32)
    W_in = np.asarray(W_in, np.float32)
    W_out = np.asarray(W_out, np.float32)
    b_out = np.asarray(b_out, np.float32)
    has_bias = bool(np.any(b_out))

    # host-side layout prep (transposes + fp16 hi/lo splits)
    qT = np.ascontiguousarray(query.transpose(0, 2, 1))        # [B, H, T]
    qh, ql = _split16(qT)
    encT = np.ascontiguousarray(enc.transpose(1, 2, 0))        # [B, H, S]
    eh, el = _split16(encT)
    en = np.ascontiguousarray(enc.transpose(1, 0, 2)).astype(np.float16)
    whf, wlf = _split16(np.ascontiguousarray(W_in.T))          # [H(h), H(i)]
    wo = np.ascontiguousarray(W_out.T).astype(np.float16)      # [2H, H]

    common = {"wh": whf, "wo": wo,
              "wh8": _f8(whf.astype(np.float32), SC_WH),
              "wl8": _f8(wlf, SC_WL)}
    if has_bias:
        common["bias"] = np.ascontiguousarray(
            np.broadcast_to(b_out[None, :], (128, H)), np.float32)

    in_maps = []
    for c in range(NCORES):
        sl = slice(c * BL, (c + 1) * BL)
        m = {
            "qh": np.ascontiguousarray(qh[sl]),
            "eh": np.ascontiguousarray(eh[sl]),
            "en": np.ascontiguousarray(en[sl]),
            "qh8": _f8(qh[sl].astype(np.float32), SC_QH),
            "ql8": _f8(ql[sl], SC_QL),
            "eh8": _f8(eh[sl].astype(np.float32), SC_EH),
            "el8": _f8(el[sl], SC_EL),
            **common,
        }
        in_maps.append(m)

    nc = _get_nc(has_bias)
    trace = bool(int(os.environ.get("KERNEL_TRACE", "0")))
    res = run_bass_kernel_spmd(nc, in_maps, core_ids=list(range(NCORES)),
                               trace=trace)
    if trace:
        _CACHE["last_exec_time_ns"] = res.exec_time_ns
        _CACHE["last_results"] = res
    out = np.concatenate([r["out"] for r in res.results], axis=0)
    return out.astype(np.float32)


# revision 9
# speedup vs baseline: 1.0756x; 1.0756x over previous
"""Trainium2 Bass kernel for nn_Attention_80779744903968.

Reference computation (B=32, T=512, S=1024, H=1024):
    z      = q @ W_in.T                  [B,T,H]
    scores = z @ enc_b.T                 [B,T,S]   (enc input is [S,B,H])
    p      = softmax(scores, axis=-1)    (the scores==0 -> -inf fill is a
                                          numerical no-op: row maxes are ~120,
                                          exp(0-max) == 0 in fp32)
    c      = p @ enc_b                   [B,T,H]
    out    = tanh([c, q] @ W_out.T + b)  [B,T,H]

Sharding: data-parallel over B across 8 cores (4 batches per core).
W_in / W_out replicated.

Precision strategy (PE matmuls):
  - z and scores need near-fp32 logits: the softmax is near-one-hot
    (logit std ~37) with near-tied rows (min top-2 gap ~2e-4), so every
    one of the four correction products matters for the absmax.
  - Both are computed as an fp16 hi/lo split: x*y ~= xh*yh (fp16 main
    pass, fp32 PSUM accumulation) + (xh*yl + xl*yh) correction passes in
    fp8(e4m3) with perf_mode=DoubleRow at 2 contraction-tiles per
    matmul; correction operands are pre-scaled by powers of 2 so both
    corr products share one PSUM scale.
  - downstream (p, enc, c, q, W_out) runs in plain fp16: p is in [0,1]
    and c/out magnitudes are O(1), so fp16's 2^-11 relative error is
    plenty.

Schedule notes (perf):
  - z and scores phases run quad-grouped: 32 fp16 main matmuls, then 32
    fp8-DR correction matmuls (pass-major).  fp16<->fp8 PE mode switches
    cost ~0.4us each, so fewer/longer runs win.
  - hi/lo merge is a single scalar_tensor_tensor reading BOTH psum banks
    (main + corr), freeing them in one DVE op.
  - softmax: exp runs on the Scalar engine with accum_out producing the
    row sum for free; p is NOT normalized -- the 1/sum scale is applied
    at the out-projection eviction (out rows have t on partitions, so
    rsum is a per-partition scalar there).  The out projection therefore
    accumulates its c-part and q-part in separate psum banks.
  - p is transposed by the DMA xbar (SBUF->SBUF, fp16) instead of the
    PE; the c matmul consumes the transposed tile directly.

All input transposes (q -> [H,T], enc -> [H,S] per batch) are done on
the host so every device-side input DMA is a contiguous natural-layout
load.  Output is written fp16 and cast to fp32 on the host.
"""
import os
import sys

import numpy as np

sys.path.insert(0, "/opt/trn_rl_repo")

import ml_dtypes  # noqa: E402

import concourse.bass as bass  # noqa: E402
import concourse.tile as tile  # noqa: E402
from concourse import bacc, mybir  # noqa: E402
from concourse.bass_utils import run_bass_kernel_spmd  # noqa: E402
from concourse.masks import make_identity  # noqa: E402

B, T, S, H = 32, 512, 1024, 1024
NCORES = 8
BL = B // NCORES  # batches per core
HT = H // 128     # h/i/k tiles per 1024
TT = T // 128     # t tiles
ST = S // 128     # s tiles
F16 = mybir.dt.float16
F32 = mybir.dt.float32
F8 = mybir.dt.float8e4
DR = mybir.MatmulPerfMode.DoubleRow

TR = os.environ.get("KERNEL_TR", "dma")      # p-transpose: dma xbar | pe
NORM = os.environ.get("KERNEL_NORM", "defer")  # softmax norm: defer | now

# power-of-2 scales for fp8 correction operands (products must share scale)
SC_WH, SC_WL, SC_QH, SC_QL = 2.0**4, 2.0**15, 1.0, 2.0**11   # z corr: 2^15
SC_ZH, SC_ZL, SC_EH, SC_EL = 1.0, 2.0**12, 1.0, 2.0**12      # s corr: 2^12

_CACHE = {}


def _build(has_bias):
    nc = bacc.Bacc("TRN2", target_bir_lowering=False, debug=False,
                   num_devices=NCORES)

    def din(name, shape, dt=F16):
        return nc.dram_tensor(name, shape, dt, kind="ExternalInput").ap()

    qh_d = din("qh", [BL, H, T])
    eh_d = din("eh", [BL, H, S])
    en_d = din("en", [BL, S, H])
    wh_d = din("wh", [H, H])
    wo_d = din("wo", [2 * H, H])
    if has_bias:
        bias_d = din("bias", [128, H], F32)
    ql8_d = din("ql8", [BL, H, T], F8)
    el8_d = din("el8", [BL, H, S], F8)
    qh8_d = din("qh8", [BL, H, T], F8)
    eh8_d = din("eh8", [BL, H, S], F8)
    wh8_d = din("wh8", [H, H], F8)
    wl8_d = din("wl8", [H, H], F8)
    out_d = nc.dram_tensor("out", [BL, T, H], F16, kind="ExternalOutput").ap()

    with tile.TileContext(nc) as tc:
        with (
            tc.tile_pool(name="weights", bufs=1) as wp,
            tc.tile_pool(name="qin", bufs=2) as qp,
            tc.tile_pool(name="ein", bufs=1) as ep,
            tc.tile_pool(name="enin", bufs=1) as enp,
            tc.tile_pool(name="zbuf", bufs=1) as zp,
            tc.tile_pool(name="scratch", bufs=2) as scrp,
            tc.tile_pool(name="scores", bufs=2) as scp,
            tc.tile_pool(name="pbuf", bufs=3) as pp,
            tc.tile_pool(name="ptbuf", bufs=1) as ptp,
            tc.tile_pool(name="ctbuf", bufs=1) as ctp,
            tc.tile_pool(name="ostage", bufs=3) as op,
            tc.tile_pool(name="stats", bufs=8) as stp,
            tc.tile_pool(name="psmm", bufs=4, space="PSUM") as psmm,
            tc.tile_pool(name="psc", bufs=2, space="PSUM") as psc,
        ):
            # --- resident weights / constants ---
            # queue order matters at startup: chunk wh/qh0 by ht so the first
            # z matmul waits on ~384KB, not 3MB (DMA sems are per-transfer);
            # z corr pass 1 needs wl8+qh8 so those precede wh8/ql8.
            wh_t = wp.tile([128, HT, H], F16)
            wh_r = wh_d.rearrange("(ht p) i -> p ht i", p=128)
            qh_first = qp.tile([128, HT, T], F16, tag="qh")
            qh_r = qh_d[0].rearrange("(ht p) t -> p ht t", p=128)
            for ht in range(HT):
                nc.sync.dma_start(wh_t[:, ht, :], wh_r[:, ht, :])
                nc.sync.dma_start(qh_first[:, ht, :], qh_r[:, ht, :])
            wl8_t = wp.tile([128, HT, H], F8)
            nc.sync.dma_start(
                wl8_t[:], wl8_d.rearrange("(ht p) i -> p ht i", p=128))
            qh8_first = qp.tile([128, HT, T], F8, tag="qh8", bufs=1)
            nc.sync.dma_start(
                qh8_first[:], qh8_d[0].rearrange("(ht p) t -> p ht t", p=128))
            wh8_t = wp.tile([128, HT, H], F8)
            nc.sync.dma_start(
                wh8_t[:], wh8_d.rearrange("(ht p) i -> p ht i", p=128))
            ql8_first = qp.tile([128, HT, T], F8, tag="ql8", bufs=1)
            nc.sync.dma_start(
                ql8_first[:], ql8_d[0].rearrange("(ht p) t -> p ht t", p=128))
            if TR == "pe":
                ident = wp.tile([128, 128], F16)
                make_identity(nc, ident[:])
            wo_t = None
            bias_t = None

            for b in range(BL):
                if b == 0:
                    qh_t, qh8_t, ql8_t = qh_first, qh8_first, ql8_first
                else:
                    qh_t = qp.tile([128, HT, T], F16, tag="qh")
                    nc.sync.dma_start(
                        qh_t[:], qh_d[b].rearrange("(ht p) t -> p ht t", p=128))
                    qh8_t = qp.tile([128, HT, T], F8, tag="qh8", bufs=1)
                    nc.sync.dma_start(
                        qh8_t[:], qh8_d[b].rearrange("(ht p) t -> p ht t", p=128))
                    ql8_t = qp.tile([128, HT, T], F8, tag="ql8", bufs=1)
                    nc.sync.dma_start(
                        ql8_t[:], ql8_d[b].rearrange("(ht p) t -> p ht t", p=128))
                eh_t = ep.tile([128, HT, S], F16, tag="eh")
                nc.sync.dma_start(
                    eh_t[:], eh_d[b].rearrange("(it p) s -> p it s", p=128))
                eh8_t = ep.tile([128, HT, S], F8, tag="eh8")
                nc.sync.dma_start(
                    eh8_t[:], eh8_d[b].rearrange("(it p) s -> p it s", p=128))
                el8_t = ep.tile([128, HT, S], F8, tag="el8")
                nc.sync.dma_start(
                    el8_t[:], el8_d[b].rearrange("(it p) s -> p it s", p=128))
                en_t = enp.tile([128, ST, H], F16, tag="en")
                nc.sync.dma_start(
                    en_t[:], en_d[b].rearrange("(st p) k -> p st k", p=128))
                if wo_t is None:
                    # deferred: b0's z/scores inputs take queue priority
                    wo_t = wp.tile([128, 2 * HT, H], F16)
                    nc.sync.dma_start(
                        wo_t[:], wo_d.rearrange("(kt p) h -> p kt h", p=128))
                    if has_bias:
                        bias_t = wp.tile([128, H], F32)
                        nc.sync.dma_start(bias_t[:], bias_d)

                # --- zT = W_inT.T @ qT (hi/lo) -> zh (f16) + fp8 lo forms ---
                zh_t = zp.tile([128, HT, T], F16, tag="zh")
                zh8_t = zp.tile([128, HT, T], F8, tag="zh8")
                zl8_t = zp.tile([128, HT, T], F8, tag="zl8")

                for quad in range(HT // 4):
                    its = range(quad * 4, quad * 4 + 4)
                    zpss = {it: psmm.tile([128, T], F32, tag="mm",
                                          name=f"zps{it % 4}")
                            for it in its}
                    # main: 32 fp16 matmuls, ht-major so b0 overlaps the
                    # chunked wh/qh arrival
                    for ht in range(HT):
                        for it in its:
                            nc.tensor.matmul(
                                zpss[it][:],
                                wh_t[:, ht, it * 128:(it + 1) * 128],
                                qh_t[:, ht, :],
                                start=(ht == 0), stop=(ht == HT - 1))
                    # corr: 32 fp8-DR matmuls, it-major (2 rotating psum
                    # banks; merge of it frees the bank for it+2)
                    zcorrs = {it: psc.tile([128, T], F32, tag="mmc",
                                           name=f"zcorr{it % 2}")
                             for it in its}
                    for it in its:
                        for pi, (lhs, rhs) in enumerate(
                                ((wl8_t, qh8_t), (wh8_t, ql8_t))):
                            for k in range(HT // 2):
                                nc.tensor.matmul(
                                    zcorrs[it][:],
                                    lhs[:, 2 * k:2 * k + 2,
                                        it * 128:(it + 1) * 128],
                                    rhs[:, 2 * k:2 * k + 2, :],
                                    start=(pi == 0 and k == 0),
                                    stop=(pi == 1 and k == HT // 2 - 1),
                                    perf_mode=DR)
                    # merge: comb = zps + zcorr * 2^-19 (DVE can read only one
                    # PSUM operand per op), then split into f16 + scaled f8 lo
                    for it in its:
                        comb = scrp.tile([128, T], F32, tag="comb")
                        nc.vector.tensor_copy(comb[:], zpss[it][:])
                        nc.vector.scalar_tensor_tensor(
                            out=comb[:], in0=zcorrs[it][:],
                            scalar=1.0 / (SC_WH * SC_QL), in1=comb[:],
                            op0=mybir.AluOpType.mult, op1=mybir.AluOpType.add)
                        nc.vector.tensor_copy(zh_t[:, it, :], comb[:])
                        zl_tmp = scrp.tile([128, T], F16, tag="zltmp")
                        nc.vector.tensor_sub(zl_tmp[:], comb[:],
                                             zh_t[:, it, :])
                        nc.vector.tensor_scalar_mul(
                            zl8_t[:, it, :], zl_tmp[:], SC_ZL)
                        nc.vector.tensor_copy(zh8_t[:, it, :], zh_t[:, it, :])

                # --- scores + softmax -> p (f16, unnormalized) ---
                # units u = (tt, sc): quad = 2 tt x 2 sc so softmax for a tt
                # can start right after its quad merges.
                p_tiles = []
                rsums = []
                sc_tiles = {}
                pt_t = ptp.tile([128, ST, T], F16, tag="pt")
                for quad in range(2):
                    units = [(quad * 2 + dt, sc) for dt in range(2)
                             for sc in range(2)]
                    spss = {}
                    for (tt, sc) in units:
                        if sc == 0:
                            sc_tiles[tt] = scp.tile([128, S], F32, tag="sc",
                                                    name=f"sc{tt % 2}")
                        sps = psmm.tile([128, 512], F32, tag="mm",
                                        name=f"sps{tt % 2}_{sc}")
                        spss[(tt, sc)] = sps
                        for it in range(HT):
                            nc.tensor.matmul(
                                sps[:],
                                zh_t[:, it, tt * 128:(tt + 1) * 128],
                                eh_t[:, it, sc * 512:(sc + 1) * 512],
                                start=(it == 0), stop=(it == HT - 1))
                    scorrs = {}
                    for ui, (tt, sc) in enumerate(units):
                        scorrs[(tt, sc)] = psc.tile([128, 512], F32, tag="mmc",
                                                    name=f"scorr{ui % 2}")
                        for pi, (lhs, rhs) in enumerate(
                                ((zl8_t, eh8_t), (zh8_t, el8_t))):
                            for k in range(HT // 2):
                                nc.tensor.matmul(
                                    scorrs[(tt, sc)][:],
                                    lhs[:, 2 * k:2 * k + 2,
                                        tt * 128:(tt + 1) * 128],
                                    rhs[:, 2 * k:2 * k + 2,
                                        sc * 512:(sc + 1) * 512],
                                    start=(pi == 0 and k == 0),
                                    stop=(pi == 1 and k == HT // 2 - 1),
                                    perf_mode=DR)
                    for ui, (tt, sc) in enumerate(units):
                        chunk = sc_tiles[tt][:, sc * 512:(sc + 1) * 512]
                        nc.vector.tensor_copy(chunk, spss[(tt, sc)][:])
                        nc.vector.scalar_tensor_tensor(
                            out=chunk, in0=scorrs[(tt, sc)][:],
                            scalar=1.0 / (SC_ZL * SC_EH),
                            in1=chunk,
                            op0=mybir.AluOpType.mult,
                            op1=mybir.AluOpType.add)
                        if sc == 1:
                            # both chunks of tt merged: softmax over free dim
                            sc_t = sc_tiles[tt]
                            negmax = stp.tile([128, 1], F32, tag="nm")
                            nc.vector.reduce_max(
                                out=negmax[:], in_=sc_t[:],
                                axis=mybir.AxisListType.X, negate=True)
                            p_t = pp.tile([128, S], F16, tag="p")
                            ssum = stp.tile([128, 1], F32, tag="ss")
                            nc.scalar.activation(
                                out=p_t[:], in_=sc_t[:],
                                func=mybir.ActivationFunctionType.Exp,
                                bias=negmax[:], scale=1.0,
                                accum_out=ssum[:])
                            rsum = stp.tile([128, 1], F32, tag="rs", bufs=8)
                            nc.vector.reciprocal(rsum[:], ssum[:])
                            rsums.append(rsum)
                            if NORM == "now":
                                nc.vector.tensor_scalar_mul(
                                    p_t[:], p_t[:], rsum[:])
                            p_tiles.append(p_t)
                            # transpose p -> pT [s, t] via DMA xbar
                            if TR == "dma":
                                for st in range(ST):
                                    nc.sync.dma_start(
                                        pt_t[:, st, tt * 128:(tt + 1) * 128],
                                        p_t[:, st * 128:(st + 1) * 128],
                                        transpose=True)

                if TR == "pe":
                    for tt in range(TT):
                        for st in range(ST):
                            tps = psc.tile([128, 128], F16, tag="tr", bufs=2)
                            nc.tensor.transpose(
                                tps[:], p_tiles[tt][:, st * 128:(st + 1) * 128],
                                ident[:])
                            nc.vector.tensor_copy(
                                pt_t[:, st, tt * 128:(tt + 1) * 128], tps[:])

                # --- cT = enc_nat.T @ pT -> [k, t] f16 (unnormalized) ---
                ct_t = ctp.tile([128, HT, T], F16, tag="ct")
                for kt in range(HT):
                    cps = psmm.tile([128, T], F32, tag="mm")
                    for st in range(ST):
                        nc.tensor.matmul(
                            cps[:],
                            en_t[:, st, kt * 128:(kt + 1) * 128],
                            pt_t[:, st, :],
                            start=(st == 0), stop=(st == ST - 1))
                    nc.vector.tensor_copy(ct_t[:, kt, :], cps[:])

                # --- out = tanh(cT.T @ WcT * rsum + qT.T @ WqT + b) ---
                for tt in range(TT):
                    for hc in range(2):
                        # q-part first: gives tail cT evictions extra slack
                        oq = psc.tile([128, 512], F32, tag="mmc")
                        for ht in range(HT):
                            nc.tensor.matmul(
                                oq[:],
                                qh_t[:, ht, tt * 128:(tt + 1) * 128],
                                wo_t[:, HT + ht, hc * 512:(hc + 1) * 512],
                                start=(ht == 0), stop=(ht == HT - 1))
                        oc = psmm.tile([128, 512], F32, tag="mm")
                        for kt in range(HT):
                            nc.tensor.matmul(
                                oc[:],
                                ct_t[:, kt, tt * 128:(tt + 1) * 128],
                                wo_t[:, kt, hc * 512:(hc + 1) * 512],
                                start=(kt == 0), stop=(kt == HT - 1))
                        ost = op.tile([128, 512], F32 if has_bias else F16,
                                      tag="os")
                        if NORM == "defer":
                            ocn = op.tile([128, 512], F32, tag="ocn")
                            nc.vector.tensor_scalar_mul(
                                ocn[:], oc[:], rsums[tt][:])
                            nc.vector.tensor_add(ost[:], ocn[:], oq[:])
                        else:
                            nc.vector.tensor_add(ost[:], oc[:], oq[:])
                        if has_bias:
                            ost16 = op.tile([128, 512], F16, tag="os16")
                            nc.vector.tensor_add(
                                ost[:], ost[:],
                                bias_t[:, hc * 512:(hc + 1) * 512])
                            nc.scalar.activation(
                                out=ost16[:], in_=ost[:],
                                func=mybir.ActivationFunctionType.Tanh)
                            ost = ost16
                        else:
                            nc.scalar.activation(
                                out=ost[:], in_=ost[:],
                                func=mybir.ActivationFunctionType.Tanh)
                        nc.sync.dma_start(
                            out_d[b, tt * 128:(tt + 1) * 128,
                                  hc * 512:(hc + 1) * 512],
                            ost[:])

    nc.compile()
    return nc


def _get_nc(has_bias):
    key = ("nc", has_bias, TR, NORM)
    if key not in _CACHE:
        _CACHE[key] = _build(has_bias)
    return _CACHE[key]


def _split16(x):
    hi = x.astype(np.float16)
    lo = (x - hi.astype(np.float32)).astype(np.float32)
    return hi, lo


def _f8(x, scale):
    return (np.asarray(x, np.float32) * np.float32(scale)).astype(
        ml_dtypes.float8_e4m3)


def kernel(query, encoder_outputs, src_lengths, W_in, W_out, b_out):
    query = np.asarray(query, np.float32)
    enc = np.asarray(encoder_outputs, np.float32)
    W_in = np.asarray(W_in, np.float32)
    W_out = np.asarray(W_out, np.float32)
    b_out = np.asarray(b_out, np.float32)
    has_bias = bool(np.any(b_out))

    # host-side layout prep (transposes + fp16 hi/lo splits)
    qT = np.ascontiguousarray(query.transpose(0, 2, 1))        # [B, H, T]
    qh, ql = _split16(qT)
    encT = np.ascontiguousarray(enc.transpose(1, 2, 0))        # [B, H, S]
    eh, el = _split16(encT)
    en = np.ascontiguousarray(enc.transpose(1, 0, 2)).astype(np.float16)
    whf, wlf = _split16(np.ascontiguousarray(W_in.T))          # [H(h), H(i)]
    wo = np.ascontiguousarray(W_out.T).astype(np.float16)      # [2H, H]

    common = {"wh": whf, "wo": wo,
              "wh8": _f8(whf.astype(np.float32), SC_WH),
              "wl8": _f8(wlf, SC_WL)}
    if has_bias:
        common["bias"] = np.ascontiguousarray(
            np.broadcast_to(b_out[None, :], (128, H)), np.float32)

    in_maps = []
    for c in range(NCORES):
        sl = slice(c * BL, (c + 1) * BL)
        m = {
            "qh": np.ascontiguousarray(qh[sl]),
            "eh": np.ascontiguousarray(eh[sl]),
            "en": np.ascontiguousarray(en[sl]),
            "qh8": _f8(qh[sl].astype(np.float32), SC_QH),
            "ql8": _f8(ql[sl], SC_QL),
            "eh8": _f8(eh[sl].astype(np.float32), SC_EH),
            "el8": _f8(el[sl], SC_EL),
            **common,
        }
        in_maps.append(m)

    nc = _get_nc(has_bias)
    trace = bool(int(os.environ.get("KERNEL_TRACE", "0")))
    res = run_bass_kernel_spmd(nc, in_maps, core_ids=list(range(NCORES)),
                               trace=trace)
    if trace:
        _CACHE["last_exec_time_ns"] = res.exec_time_ns
        _CACHE["last_results"] = res
    out = np.concatenate([r["out"] for r in res.results], axis=0)
    return out.astype(np.float32)


# revision 13
# speedup vs baseline: 1.2575x; 1.1691x over previous
"""Trainium2 Bass kernel for nn_Attention_80779744903968.

Reference computation (B=32, T=512, S=1024, H=1024):
    z      = q @ W_in.T                  [B,T,H]
    scores = z @ enc_b.T                 [B,T,S]   (enc input is [S,B,H])
    p      = softmax(scores, axis=-1)    (the scores==0 -> -inf fill is a
                                          numerical no-op: row maxes are ~120,
                                          exp(0-max) == 0 in fp32)
    c      = p @ enc_b                   [B,T,H]
    out    = tanh([c, q] @ W_out.T + b)  [B,T,H]

Sharding: data-parallel over B across 8 cores (4 batches per core).
W_in / W_out replicated.

Precision strategy (PE matmuls):
  - z and scores need near-fp32 logits: the softmax is near-one-hot
    (logit std ~37) with near-tied rows (min top-2 gap ~2e-4), so every
    one of the four correction products matters for the absmax.
  - Both are computed as an fp16 hi/lo split: x*y ~= xh*yh (fp16 main
    pass, fp32 PSUM accumulation) + (xh*yl + xl*yh) correction passes in
    fp8(e4m3) with perf_mode=DoubleRow at 2 contraction-tiles per
    matmul; correction operands are pre-scaled by powers of 2 so both
    corr products share one PSUM scale.
  - downstream (p, enc, c, q, W_out) runs in plain fp16: p is in [0,1]
    and c/out magnitudes are O(1), so fp16's 2^-11 relative error is
    plenty.

Schedule notes (perf):
  - z and scores phases run quad-grouped: 32 fp16 main matmuls, then 32
    fp8-DR correction matmuls (pass-major).  fp16<->fp8 PE mode switches
    cost ~0.4us each, so fewer/longer runs win.
  - hi/lo merge is a single scalar_tensor_tensor reading BOTH psum banks
    (main + corr), freeing them in one DVE op.
  - softmax: exp runs on the Scalar engine with accum_out producing the
    row sum for free; p is NOT normalized -- the 1/sum scale is applied
    at the out-projection eviction (out rows have t on partitions, so
    rsum is a per-partition scalar there).  The out projection therefore
    accumulates its c-part and q-part in separate psum banks.
  - p is transposed by the DMA xbar (SBUF->SBUF, fp16) instead of the
    PE; the c matmul consumes the transposed tile directly.

All input transposes (q -> [H,T], enc -> [H,S] per batch) are done on
the host so every device-side input DMA is a contiguous natural-layout
load.  Output is written fp16 and cast to fp32 on the host.
"""
import os
import sys

import numpy as np

sys.path.insert(0, "/opt/trn_rl_repo")

import ml_dtypes  # noqa: E402

import concourse.bass as bass  # noqa: E402
import concourse.tile as tile  # noqa: E402
from concourse import bacc, mybir  # noqa: E402
from concourse.bass_utils import run_bass_kernel_spmd  # noqa: E402
from concourse.masks import make_identity  # noqa: E402

B, T, S, H = 32, 512, 1024, 1024
NCORES = 8
BL = B // NCORES  # batches per core
HT = H // 128     # h/i/k tiles per 1024
TT = T // 128     # t tiles
ST = S // 128     # s tiles
F16 = mybir.dt.float16
F32 = mybir.dt.float32
F8 = mybir.dt.float8e4
DR = mybir.MatmulPerfMode.DoubleRow

TR = os.environ.get("KERNEL_TR", "dma")      # p-transpose: dma xbar | pe
NORM = os.environ.get("KERNEL_NORM", "defer")  # softmax norm: defer | now

# power-of-2 scales for fp8 correction operands (products must share scale)
SC_WH, SC_WL, SC_QH, SC_QL = 2.0**4, 2.0**15, 1.0, 2.0**11   # z corr: 2^15
SC_ZH, SC_ZL, SC_EH, SC_EL = 1.0, 2.0**12, 1.0, 2.0**12      # s corr: 2^12

_CACHE = {}


def _build(has_bias):
    nc = bacc.Bacc("TRN2", target_bir_lowering=False, debug=False,
                   num_devices=NCORES)

    def din(name, shape, dt=F16):
        return nc.dram_tensor(name, shape, dt, kind="ExternalInput").ap()

    qh_d = din("qh", [BL, H, T])
    eh_d = din("eh", [BL, H, S])
    en_d = din("en", [BL, S, H])
    wh_d = din("wh", [H, H])
    wo_d = din("wo", [2 * H, H])
    if has_bias:
        bias_d = din("bias", [128, H], F32)
    ql8_d = din("ql8", [BL, H, T], F8)
    el8_d = din("el8", [BL, H, S], F8)
    qh8_d = din("qh8", [BL, H, T], F8)
    eh8_d = din("eh8", [BL, H, S], F8)
    wh8_d = din("wh8", [H, H], F8)
    wl8_d = din("wl8", [H, H], F8)
    out_d = nc.dram_tensor("out", [BL, T, H], F16, kind="ExternalOutput").ap()

    with tile.TileContext(nc) as tc:
        with (
            tc.tile_pool(name="weights", bufs=1) as wp,
            tc.tile_pool(name="qin", bufs=2) as qp,
            tc.tile_pool(name="ein", bufs=1) as ep,
            tc.tile_pool(name="enin", bufs=1) as enp,
            tc.tile_pool(name="zbuf", bufs=1) as zp,
            tc.tile_pool(name="scratch", bufs=2) as scrp,
            tc.tile_pool(name="scores", bufs=2) as scp,
            tc.tile_pool(name="pbuf", bufs=3) as pp,
            tc.tile_pool(name="ptbuf", bufs=1) as ptp,
            tc.tile_pool(name="ctbuf", bufs=1) as ctp,
            tc.tile_pool(name="ostage", bufs=3) as op,
            tc.tile_pool(name="stats", bufs=8) as stp,
            tc.tile_pool(name="psmm", bufs=4, space="PSUM") as psmm,
            tc.tile_pool(name="psc", bufs=2, space="PSUM") as psc,
        ):
            # --- resident weights / constants ---
            # queue order matters at startup: chunk wh/qh0 by ht so the first
            # z matmul waits on ~384KB, not 3MB (DMA sems are per-transfer);
            # z corr pass 1 needs wl8+qh8 so those precede wh8/ql8.
            wh_t = wp.tile([128, HT, H], F16)
            wh_r = wh_d.rearrange("(ht p) i -> p ht i", p=128)
            qh_first = qp.tile([128, HT, T], F16, tag="qh")
            qh_r = qh_d[0].rearrange("(ht p) t -> p ht t", p=128)
            for ht in range(HT):
                nc.sync.dma_start(wh_t[:, ht, :], wh_r[:, ht, :])
                nc.sync.dma_start(qh_first[:, ht, :], qh_r[:, ht, :])
            wl8_t = wp.tile([128, HT, H], F8)
            nc.sync.dma_start(
                wl8_t[:], wl8_d.rearrange("(ht p) i -> p ht i", p=128))
            qh8_first = qp.tile([128, HT, T], F8, tag="qh8", bufs=1)
            nc.sync.dma_start(
                qh8_first[:], qh8_d[0].rearrange("(ht p) t -> p ht t", p=128))
            wh8_t = wp.tile([128, HT, H], F8)
            nc.sync.dma_start(
                wh8_t[:], wh8_d.rearrange("(ht p) i -> p ht i", p=128))
            ql8_first = qp.tile([128, HT, T], F8, tag="ql8", bufs=1)
            nc.sync.dma_start(
                ql8_first[:], ql8_d[0].rearrange("(ht p) t -> p ht t", p=128))
            if TR == "pe":
                ident = wp.tile([128, 128], F16)
                make_identity(nc, ident[:])
            wo_t = None
            bias_t = None

            for b in range(BL):
                if b == 0:
                    qh_t, qh8_t, ql8_t = qh_first, qh8_first, ql8_first
                else:
                    qh_t = qp.tile([128, HT, T], F16, tag="qh")
                    nc.sync.dma_start(
                        qh_t[:], qh_d[b].rearrange("(ht p) t -> p ht t", p=128))
                    qh8_t = qp.tile([128, HT, T], F8, tag="qh8", bufs=1)
                    nc.sync.dma_start(
                        qh8_t[:], qh8_d[b].rearrange("(ht p) t -> p ht t", p=128))
                    ql8_t = qp.tile([128, HT, T], F8, tag="ql8", bufs=1)
                    nc.sync.dma_start(
                        ql8_t[:], ql8_d[b].rearrange("(ht p) t -> p ht t", p=128))
                eh_t = ep.tile([128, HT, S], F16, tag="eh")
                nc.sync.dma_start(
                    eh_t[:], eh_d[b].rearrange("(it p) s -> p it s", p=128))
                eh8_t = ep.tile([128, HT, S], F8, tag="eh8")
                nc.sync.dma_start(
                    eh8_t[:], eh8_d[b].rearrange("(it p) s -> p it s", p=128))
                el8_t = ep.tile([128, HT, S], F8, tag="el8")
                nc.sync.dma_start(
                    el8_t[:], el8_d[b].rearrange("(it p) s -> p it s", p=128))
                en_t = enp.tile([128, ST, H], F16, tag="en")
                nc.sync.dma_start(
                    en_t[:], en_d[b].rearrange("(st p) k -> p st k", p=128))
                if wo_t is None:
                    # deferred: b0's z/scores inputs take queue priority
                    wo_t = wp.tile([128, 2 * HT, H], F16)
                    nc.sync.dma_start(
                        wo_t[:], wo_d.rearrange("(kt p) h -> p kt h", p=128))
                    if has_bias:
                        bias_t = wp.tile([128, H], F32)
                        nc.sync.dma_start(bias_t[:], bias_d)

                # --- zT = W_inT.T @ qT (hi/lo) -> zh (f16) + fp8 lo forms ---
                zh_t = zp.tile([128, HT, T], F16, tag="zh")
                zh8_t = zp.tile([128, HT, T], F8, tag="zh8")
                zl8_t = zp.tile([128, HT, T], F8, tag="zl8")

                for quad in range(HT // 4):
                    its = range(quad * 4, quad * 4 + 4)
                    zpss = {it: psmm.tile([128, T], F32, tag="mm",
                                          name=f"zps{it % 4}")
                            for it in its}
                    # main: 32 fp16 matmuls, ht-major so b0 overlaps the
                    # chunked wh/qh arrival
                    for ht in range(HT):
                        for it in its:
                            nc.tensor.matmul(
                                zpss[it][:],
                                wh_t[:, ht, it * 128:(it + 1) * 128],
                                qh_t[:, ht, :],
                                start=(ht == 0), stop=(ht == HT - 1))
                    # corr: 32 fp8-DR matmuls, it-major (2 rotating psum
                    # banks; merge of it frees the bank for it+2)
                    zcorrs = {it: psc.tile([128, T], F32, tag="mmc",
                                           name=f"zcorr{it % 2}")
                             for it in its}
                    for it in its:
                        for pi, (lhs, rhs) in enumerate(
                                ((wl8_t, qh8_t), (wh8_t, ql8_t))):
                            for k in range(HT // 2):
                                nc.tensor.matmul(
                                    zcorrs[it][:],
                                    lhs[:, 2 * k:2 * k + 2,
                                        it * 128:(it + 1) * 128],
                                    rhs[:, 2 * k:2 * k + 2, :],
                                    start=(pi == 0 and k == 0),
                                    stop=(pi == 1 and k == HT // 2 - 1),
                                    perf_mode=DR)
                    # merge: comb = zps + zcorr * 2^-19 (DVE can read only one
                    # PSUM operand per op), then split into f16 + scaled f8
                    # lo.  The casts run on the Scalar engine so DVE (copy +
                    # STT + sub = 1.75us/it) keeps up with the PE's corr pace
                    # (2.0us/it) and psum banks recycle without stalling.
                    for it in its:
                        comb = scrp.tile([128, T], F32, tag="comb")
                        nc.vector.tensor_copy(comb[:], zpss[it][:])
                        nc.vector.scalar_tensor_tensor(
                            out=comb[:], in0=zcorrs[it][:],
                            scalar=1.0 / (SC_WH * SC_QL), in1=comb[:],
                            op0=mybir.AluOpType.mult, op1=mybir.AluOpType.add)
                        nc.scalar.activation(
                            out=zh_t[:, it, :], in_=comb[:],
                            func=mybir.ActivationFunctionType.Copy)
                        zl_tmp = scrp.tile([128, T], F16, tag="zltmp")
                        nc.vector.tensor_sub(zl_tmp[:], comb[:],
                                             zh_t[:, it, :])
                        nc.scalar.activation(
                            out=zl8_t[:, it, :], in_=zl_tmp[:],
                            func=mybir.ActivationFunctionType.Copy,
                            scale=SC_ZL)
                        nc.scalar.activation(
                            out=zh8_t[:, it, :], in_=zh_t[:, it, :],
                            func=mybir.ActivationFunctionType.Copy)

                # --- scores + softmax -> p (f16, unnormalized) ---
                # units u = (tt, sc): quad = 2 tt x 2 sc so softmax for a tt
                # can start right after its quad merges.
                p_tiles = []
                rsums = []
                sc_tiles = {}
                pt_t = ptp.tile([128, ST, T], F16, tag="pt")
                for quad in range(2):
                    units = [(quad * 2 + dt, sc) for dt in range(2)
                             for sc in range(2)]
                    spss = {}
                    for (tt, sc) in units:
                        if sc == 0:
                            sc_tiles[tt] = scp.tile([128, S], F32, tag="sc",
                                                    name=f"sc{tt % 2}")
                        sps = psmm.tile([128, 512], F32, tag="mm",
                                        name=f"sps{tt % 2}_{sc}")
                        spss[(tt, sc)] = sps
                        for it in range(HT):
                            nc.tensor.matmul(
                                sps[:],
                                zh_t[:, it, tt * 128:(tt + 1) * 128],
                                eh_t[:, it, sc * 512:(sc + 1) * 512],
                                start=(it == 0), stop=(it == HT - 1))
                    scorrs = {}
                    for ui, (tt, sc) in enumerate(units):
                        scorrs[(tt, sc)] = psc.tile([128, 512], F32, tag="mmc",
                                                    name=f"scorr{ui % 2}")
                        for pi, (lhs, rhs) in enumerate(
                                ((zl8_t, eh8_t), (zh8_t, el8_t))):
                            for k in range(HT // 2):
                                nc.tensor.matmul(
                                    scorrs[(tt, sc)][:],
                                    lhs[:, 2 * k:2 * k + 2,
                                        tt * 128:(tt + 1) * 128],
                                    rhs[:, 2 * k:2 * k + 2,
                                        sc * 512:(sc + 1) * 512],
                                    start=(pi == 0 and k == 0),
                                    stop=(pi == 1 and k == HT // 2 - 1),
                                    perf_mode=DR)
                    for ui, (tt, sc) in enumerate(units):
                        chunk = sc_tiles[tt][:, sc * 512:(sc + 1) * 512]
                        nc.scalar.activation(
                            out=chunk, in_=spss[(tt, sc)][:],
                            func=mybir.ActivationFunctionType.Copy)
                        nc.vector.scalar_tensor_tensor(
                            out=chunk, in0=scorrs[(tt, sc)][:],
                            scalar=1.0 / (SC_ZL * SC_EH),
                            in1=chunk,
                            op0=mybir.AluOpType.mult,
                            op1=mybir.AluOpType.add)
                        if sc == 1:
                            # both chunks of tt merged: softmax over free dim
                            sc_t = sc_tiles[tt]
                            negmax = stp.tile([128, 1], F32, tag="nm")
                            nc.vector.reduce_max(
                                out=negmax[:], in_=sc_t[:],
                                axis=mybir.AxisListType.X, negate=True)
                            p_t = pp.tile([128, S], F16, tag="p")
                            ssum = stp.tile([128, 1], F32, tag="ss")
                            nc.scalar.activation(
                                out=p_t[:], in_=sc_t[:],
                                func=mybir.ActivationFunctionType.Exp,
                                bias=negmax[:], scale=1.0,
                                accum_out=ssum[:])
                            rsum = stp.tile([128, 1], F32, tag="rs", bufs=8)
                            nc.vector.reciprocal(rsum[:], ssum[:])
                            rsums.append(rsum)
                            if NORM == "now":
                                nc.vector.tensor_scalar_mul(
                                    p_t[:], p_t[:], rsum[:])
                            p_tiles.append(p_t)
                            # transpose p -> pT [s, t] via DMA xbar
                            if TR == "dma":
                                for st in range(ST):
                                    nc.sync.dma_start(
                                        pt_t[:, st, tt * 128:(tt + 1) * 128],
                                        p_t[:, st * 128:(st + 1) * 128],
                                        transpose=True)

                if TR == "pe":
                    for tt in range(TT):
                        for st in range(ST):
                            tps = psc.tile([128, 128], F16, tag="tr", bufs=2)
                            nc.tensor.transpose(
                                tps[:], p_tiles[tt][:, st * 128:(st + 1) * 128],
                                ident[:])
                            nc.scalar.activation(
                                out=pt_t[:, st, tt * 128:(tt + 1) * 128],
                                in_=tps[:],
                                func=mybir.ActivationFunctionType.Copy)

                # --- cT = enc_nat.T @ pT -> [k, t] f16 (unnormalized) ---
                ct_t = ctp.tile([128, HT, T], F16, tag="ct")
                for kt in range(HT):
                    cps = psmm.tile([128, T], F32, tag="mm")
                    for st in range(ST):
                        nc.tensor.matmul(
                            cps[:],
                            en_t[:, st, kt * 128:(kt + 1) * 128],
                            pt_t[:, st, :],
                            start=(st == 0), stop=(st == ST - 1))
                    nc.scalar.activation(
                        out=ct_t[:, kt, :], in_=cps[:],
                        func=mybir.ActivationFunctionType.Copy)

                # --- out = tanh(cT.T @ WcT * rsum + qT.T @ WqT + b) ---
                for tt in range(TT):
                    for hc in range(2):
                        # q-part first: gives tail cT evictions extra slack
                        oq = psc.tile([128, 512], F32, tag="mmc")
                        for ht in range(HT):
                            nc.tensor.matmul(
                                oq[:],
                                qh_t[:, ht, tt * 128:(tt + 1) * 128],
                                wo_t[:, HT + ht, hc * 512:(hc + 1) * 512],
                                start=(ht == 0), stop=(ht == HT - 1))
                        oc = psmm.tile([128, 512], F32, tag="mm")
                        for kt in range(HT):
                            nc.tensor.matmul(
                                oc[:],
                                ct_t[:, kt, tt * 128:(tt + 1) * 128],
                                wo_t[:, kt, hc * 512:(hc + 1) * 512],
                                start=(kt == 0), stop=(kt == HT - 1))
                        ost = op.tile([128, 512], F32 if has_bias else F16,
                                      tag="os")
                        if NORM == "defer":
                            ocn = op.tile([128, 512], F32, tag="ocn")
                            nc.vector.tensor_scalar_mul(
                                ocn[:], oc[:], rsums[tt][:])
                            nc.vector.tensor_add(ost[:], ocn[:], oq[:])
                        else:
                            nc.vector.tensor_add(ost[:], oc[:], oq[:])
                        if has_bias:
                            ost16 = op.tile([128, 512], F16, tag="os16")
                            nc.vector.tensor_add(
                                ost[:], ost[:],
                                bias_t[:, hc * 512:(hc + 1) * 512])
                            nc.scalar.activation(
                                out=ost16[:], in_=ost[:],
                                func=mybir.ActivationFunctionType.Tanh)
                            ost = ost16
                        else:
                            nc.scalar.activation(
                                out=ost[:], in_=ost[:],
                                func=mybir.ActivationFunctionType.Tanh)
                        nc.sync.dma_start(
                            out_d[b, tt * 128:(tt + 1) * 128,
                                  hc * 512:(hc + 1) * 512],
                            ost[:])

    nc.compile()
    return nc


def _get_nc(has_bias):
    key = ("nc", has_bias, TR, NORM)
    if key not in _CACHE:
        _CACHE[key] = _build(has_bias)
    return _CACHE[key]


def _split16(x):
    hi = x.astype(np.float16)
    lo = (x - hi.astype(np.float32)).astype(np.float32)
    return hi, lo


def _f8(x, scale):
    return (np.asarray(x, np.float32) * np.float32(scale)).astype(
        ml_dtypes.float8_e4m3)


def kernel(query, encoder_outputs, src_lengths, W_in, W_out, b_out):
    query = np.asarray(query, np.float32)
    enc = np.asarray(encoder_outputs, np.float32)
    W_in = np.asarray(W_in, np.float32)
    W_out = np.asarray(W_out, np.float32)
    b_out = np.asarray(b_out, np.float32)
    has_bias = bool(np.any(b_out))

    # host-side layout prep (transposes + fp16 hi/lo splits)
    qT = np.ascontiguousarray(query.transpose(0, 2, 1))        # [B, H, T]
    qh, ql = _split16(qT)
    encT = np.ascontiguousarray(enc.transpose(1, 2, 0))        # [B, H, S]
    eh, el = _split16(encT)
    en = np.ascontiguousarray(enc.transpose(1, 0, 2)).astype(np.float16)
    whf, wlf = _split16(np.ascontiguousarray(W_in.T))          # [H(h), H(i)]
    wo = np.ascontiguousarray(W_out.T).astype(np.float16)      # [2H, H]

    common = {"wh": whf, "wo": wo,
              "wh8": _f8(whf.astype(np.float32), SC_WH),
              "wl8": _f8(wlf, SC_WL)}
    if has_bias:
        common["bias"] = np.ascontiguousarray(
            np.broadcast_to(b_out[None, :], (128, H)), np.float32)

    in_maps = []
    for c in range(NCORES):
        sl = slice(c * BL, (c + 1) * BL)
        m = {
            "qh": np.ascontiguousarray(qh[sl]),
            "eh": np.ascontiguousarray(eh[sl]),
            "en": np.ascontiguousarray(en[sl]),
            "qh8": _f8(qh[sl].astype(np.float32), SC_QH),
            "ql8": _f8(ql[sl], SC_QL),
            "eh8": _f8(eh[sl].astype(np.float32), SC_EH),
            "el8": _f8(el[sl], SC_EL),
            **common,
        }
        in_maps.append(m)

    nc = _get_nc(has_bias)
    trace = bool(int(os.environ.get("KERNEL_TRACE", "0")))
    res = run_bass_kernel_spmd(nc, in_maps, core_ids=list(range(NCORES)),
                               trace=trace)
    if trace:
        _CACHE["last_exec_time_ns"] = res.exec_time_ns
        _CACHE["last_results"] = res
    out = np.concatenate([r["out"] for r in res.results], axis=0)
    return out.astype(np.float32)


# revision 15
# speedup vs baseline: 1.3244x; 1.0532x over previous
"""Trainium2 Bass kernel for nn_Attention_80779744903968.

Reference computation (B=32, T=512, S=1024, H=1024):
    z      = q @ W_in.T                  [B,T,H]
    scores = z @ enc_b.T                 [B,T,S]   (enc input is [S,B,H])
    p      = softmax(scores, axis=-1)    (the scores==0 -> -inf fill is a
                                          numerical no-op: row maxes are ~120,
                                          exp(0-max) == 0 in fp32)
    c      = p @ enc_b                   [B,T,H]
    out    = tanh([c, q] @ W_out.T + b)  [B,T,H]

Sharding: data-parallel over B across 8 cores (4 batches per core).
W_in / W_out replicated.

Precision strategy (PE matmuls):
  - z and scores need near-fp32 logits: the softmax is near-one-hot
    (logit std ~37) with near-tied rows (min top-2 gap ~2e-4), so every
    one of the four correction products matters for the absmax.
  - Both are computed as an fp16 hi/lo split: x*y ~= xh*yh (fp16 main
    pass, fp32 PSUM accumulation) + (xh*yl + xl*yh) correction passes in
    fp8(e4m3) with perf_mode=DoubleRow at 2 contraction-tiles per
    matmul; correction operands are pre-scaled by powers of 2 so both
    corr products share one PSUM scale.
  - downstream (p, enc, c, q, W_out) runs in plain fp16: p is in [0,1]
    and c/out magnitudes are O(1), so fp16's 2^-11 relative error is
    plenty.

Schedule notes (perf):
  - z and scores phases run quad-grouped: 32 fp16 main matmuls, then 32
    fp8-DR correction matmuls (pass-major).  fp16<->fp8 PE mode switches
    cost ~0.4us each, so fewer/longer runs win.
  - hi/lo merge is a single scalar_tensor_tensor reading BOTH psum banks
    (main + corr), freeing them in one DVE op.
  - softmax: exp runs on the Scalar engine with accum_out producing the
    row sum for free; p is NOT normalized -- the 1/sum scale is applied
    at the out-projection eviction (out rows have t on partitions, so
    rsum is a per-partition scalar there).  The out projection therefore
    accumulates its c-part and q-part in separate psum banks.
  - p is transposed by the DMA xbar (SBUF->SBUF, fp16) instead of the
    PE; the c matmul consumes the transposed tile directly.

All input transposes (q -> [H,T], enc -> [H,S] per batch) are done on
the host so every device-side input DMA is a contiguous natural-layout
load.  Output is written fp16 and cast to fp32 on the host.
"""
import os
import sys

import numpy as np

sys.path.insert(0, "/opt/trn_rl_repo")

import ml_dtypes  # noqa: E402

import concourse.bass as bass  # noqa: E402
import concourse.tile as tile  # noqa: E402
from concourse import bacc, mybir  # noqa: E402
from concourse.bass_utils import run_bass_kernel_spmd  # noqa: E402
from concourse.masks import make_identity  # noqa: E402

B, T, S, H = 32, 512, 1024, 1024
NCORES = 8
BL = B // NCORES  # batches per core
HT = H // 128     # h/i/k tiles per 1024
TT = T // 128     # t tiles
ST = S // 128     # s tiles
F16 = mybir.dt.float16
F32 = mybir.dt.float32
F8 = mybir.dt.float8e4
DR = mybir.MatmulPerfMode.DoubleRow

TR = os.environ.get("KERNEL_TR", "dma")      # p-transpose: dma xbar | pe
NORM = os.environ.get("KERNEL_NORM", "defer")  # softmax norm: defer | now

# power-of-2 scales for fp8 correction operands (products must share scale)
SC_WH, SC_WL, SC_QH, SC_QL = 2.0**4, 2.0**15, 1.0, 2.0**11   # z corr: 2^15
SC_ZH, SC_ZL, SC_EH, SC_EL = 1.0, 2.0**12, 1.0, 2.0**12      # s corr: 2^12

_CACHE = {}


def _build(has_bias):
    nc = bacc.Bacc("TRN2", target_bir_lowering=False, debug=False,
                   num_devices=NCORES)

    def din(name, shape, dt=F16):
        return nc.dram_tensor(name, shape, dt, kind="ExternalInput").ap()

    qh_d = din("qh", [BL, H, T])
    eh_d = din("eh", [BL, H, S])
    en_d = din("en", [BL, S, H])
    wh_d = din("wh", [H, H])
    wo_d = din("wo", [2 * H, H])
    if has_bias:
        bias_d = din("bias", [128, H], F32)
    ql8_d = din("ql8", [BL, H, T], F8)
    el8_d = din("el8", [BL, H, S], F8)
    qh8_d = din("qh8", [BL, H, T], F8)
    eh8_d = din("eh8", [BL, H, S], F8)
    wh8_d = din("wh8", [H, H], F8)
    wl8_d = din("wl8", [H, H], F8)
    out_d = nc.dram_tensor("out", [BL, T, H], F16, kind="ExternalOutput").ap()

    with tile.TileContext(nc) as tc:
        with (
            tc.tile_pool(name="weights", bufs=1) as wp,
            tc.tile_pool(name="qin", bufs=2) as qp,
            tc.tile_pool(name="ein", bufs=1) as ep,
            tc.tile_pool(name="enin", bufs=1) as enp,
            tc.tile_pool(name="zbuf", bufs=1) as zp,
            tc.tile_pool(name="scratch", bufs=2) as scrp,
            tc.tile_pool(name="scores", bufs=2) as scp,
            tc.tile_pool(name="pbuf", bufs=4) as pp,
            tc.tile_pool(name="ptbuf", bufs=1) as ptp,
            tc.tile_pool(name="ctbuf", bufs=1) as ctp,
            tc.tile_pool(name="ostage", bufs=3) as op,
            tc.tile_pool(name="stats", bufs=8) as stp,
            tc.tile_pool(name="psmm", bufs=4, space="PSUM") as psmm,
            tc.tile_pool(name="psc", bufs=2, space="PSUM") as psc,
        ):
            # --- resident weights / constants ---
            # queue order matters at startup: chunk wh/qh0 by ht so the first
            # z matmul waits on ~384KB, not 3MB (DMA sems are per-transfer);
            # z corr pass 1 needs wl8+qh8 so those precede wh8/ql8.
            wh_t = wp.tile([128, HT, H], F16)
            wh_r = wh_d.rearrange("(ht p) i -> p ht i", p=128)
            qh_first = qp.tile([128, HT, T], F16, tag="qh")
            qh_r = qh_d[0].rearrange("(ht p) t -> p ht t", p=128)
            for ht in range(HT):
                nc.sync.dma_start(wh_t[:, ht, :], wh_r[:, ht, :])
                nc.sync.dma_start(qh_first[:, ht, :], qh_r[:, ht, :])
            wl8_t = wp.tile([128, HT, H], F8)
            nc.sync.dma_start(
                wl8_t[:], wl8_d.rearrange("(ht p) i -> p ht i", p=128))
            qh8_first = qp.tile([128, HT, T], F8, tag="qh8", bufs=1)
            nc.sync.dma_start(
                qh8_first[:], qh8_d[0].rearrange("(ht p) t -> p ht t", p=128))
            wh8_t = wp.tile([128, HT, H], F8)
            nc.sync.dma_start(
                wh8_t[:], wh8_d.rearrange("(ht p) i -> p ht i", p=128))
            ql8_first = qp.tile([128, HT, T], F8, tag="ql8", bufs=1)
            nc.sync.dma_start(
                ql8_first[:], ql8_d[0].rearrange("(ht p) t -> p ht t", p=128))
            if TR == "pe":
                ident = wp.tile([128, 128], F16)
                make_identity(nc, ident[:])
            wo_t = None
            bias_t = None

            for b in range(BL):
                if b == 0:
                    qh_t, qh8_t, ql8_t = qh_first, qh8_first, ql8_first
                else:
                    qh_t = qp.tile([128, HT, T], F16, tag="qh")
                    nc.sync.dma_start(
                        qh_t[:], qh_d[b].rearrange("(ht p) t -> p ht t", p=128))
                    qh8_t = qp.tile([128, HT, T], F8, tag="qh8", bufs=1)
                    nc.sync.dma_start(
                        qh8_t[:], qh8_d[b].rearrange("(ht p) t -> p ht t", p=128))
                    ql8_t = qp.tile([128, HT, T], F8, tag="ql8", bufs=1)
                    nc.sync.dma_start(
                        ql8_t[:], ql8_d[b].rearrange("(ht p) t -> p ht t", p=128))
                eh_t = ep.tile([128, HT, S], F16, tag="eh")
                nc.sync.dma_start(
                    eh_t[:], eh_d[b].rearrange("(it p) s -> p it s", p=128))
                eh8_t = ep.tile([128, HT, S], F8, tag="eh8")
                nc.sync.dma_start(
                    eh8_t[:], eh8_d[b].rearrange("(it p) s -> p it s", p=128))
                el8_t = ep.tile([128, HT, S], F8, tag="el8")
                nc.sync.dma_start(
                    el8_t[:], el8_d[b].rearrange("(it p) s -> p it s", p=128))
                en_t = enp.tile([128, ST, H], F16, tag="en")
                nc.sync.dma_start(
                    en_t[:], en_d[b].rearrange("(st p) k -> p st k", p=128))
                if wo_t is None:
                    # deferred: b0's z/scores inputs take queue priority
                    wo_t = wp.tile([128, 2 * HT, H], F16)
                    nc.sync.dma_start(
                        wo_t[:], wo_d.rearrange("(kt p) h -> p kt h", p=128))
                    if has_bias:
                        bias_t = wp.tile([128, H], F32)
                        nc.sync.dma_start(bias_t[:], bias_d)

                # --- zT = W_inT.T @ qT (hi/lo) -> zh (f16) + fp8 lo forms ---
                zh_t = zp.tile([128, HT, T], F16, tag="zh")
                zh8_t = zp.tile([128, HT, T], F8, tag="zh8")
                zl8_t = zp.tile([128, HT, T], F8, tag="zl8")

                for quad in range(HT // 4):
                    its = range(quad * 4, quad * 4 + 4)
                    zpss = {it: psmm.tile([128, T], F32, tag="mm",
                                          name=f"zps{it % 4}")
                            for it in its}
                    # main: 32 fp16 matmuls, ht-major so b0 overlaps the
                    # chunked wh/qh arrival
                    for ht in range(HT):
                        for it in its:
                            nc.tensor.matmul(
                                zpss[it][:],
                                wh_t[:, ht, it * 128:(it + 1) * 128],
                                qh_t[:, ht, :],
                                start=(ht == 0), stop=(ht == HT - 1))
                    # corr: 32 fp8-DR matmuls, it-major (2 rotating psum
                    # banks; merge of it frees the bank for it+2)
                    zcorrs = {it: psc.tile([128, T], F32, tag="mmc",
                                           name=f"zcorr{it % 2}")
                             for it in its}
                    for it in its:
                        for pi, (lhs, rhs) in enumerate(
                                ((wl8_t, qh8_t), (wh8_t, ql8_t))):
                            for k in range(HT // 2):
                                nc.tensor.matmul(
                                    zcorrs[it][:],
                                    lhs[:, 2 * k:2 * k + 2,
                                        it * 128:(it + 1) * 128],
                                    rhs[:, 2 * k:2 * k + 2, :],
                                    start=(pi == 0 and k == 0),
                                    stop=(pi == 1 and k == HT // 2 - 1),
                                    perf_mode=DR)
                    # merge: comb = zps + zcorr * 2^-19 (DVE can read only one
                    # PSUM operand per op), then split into f16 + scaled f8
                    # lo.  The casts run on the Scalar engine so DVE (copy +
                    # STT + sub = 1.75us/it) keeps up with the PE's corr pace
                    # (2.0us/it) and psum banks recycle without stalling.
                    for it in its:
                        comb = scrp.tile([128, T], F32, tag="comb")
                        nc.vector.tensor_copy(comb[:], zpss[it][:])
                        nc.vector.scalar_tensor_tensor(
                            out=comb[:], in0=zcorrs[it][:],
                            scalar=1.0 / (SC_WH * SC_QL), in1=comb[:],
                            op0=mybir.AluOpType.mult, op1=mybir.AluOpType.add)
                        nc.scalar.activation(
                            out=zh_t[:, it, :], in_=comb[:],
                            func=mybir.ActivationFunctionType.Copy)
                        zl_tmp = scrp.tile([128, T], F16, tag="zltmp")
                        nc.vector.tensor_sub(zl_tmp[:], comb[:],
                                             zh_t[:, it, :])
                        nc.scalar.activation(
                            out=zl8_t[:, it, :], in_=zl_tmp[:],
                            func=mybir.ActivationFunctionType.Copy,
                            scale=SC_ZL)
                        nc.scalar.activation(
                            out=zh8_t[:, it, :], in_=zh_t[:, it, :],
                            func=mybir.ActivationFunctionType.Copy)

                # --- scores + softmax -> p (f16, unnormalized) ---
                # units u = (tt, sc): quad = 2 tt x 2 sc so softmax for a tt
                # can start right after its quad merges.
                p_tiles = []
                rsums = []
                sc_tiles = {}
                pt_t = ptp.tile([128, ST, T], F16, tag="pt")
                for quad in range(2):
                    units = [(quad * 2 + dt, sc) for dt in range(2)
                             for sc in range(2)]
                    spss = {}
                    for (tt, sc) in units:
                        if sc == 0:
                            sc_tiles[tt] = scp.tile([128, S], F32, tag="sc",
                                                    name=f"sc{tt % 2}")
                        sps = psmm.tile([128, 512], F32, tag="mm",
                                        name=f"sps{tt % 2}_{sc}")
                        spss[(tt, sc)] = sps
                        for it in range(HT):
                            nc.tensor.matmul(
                                sps[:],
                                zh_t[:, it, tt * 128:(tt + 1) * 128],
                                eh_t[:, it, sc * 512:(sc + 1) * 512],
                                start=(it == 0), stop=(it == HT - 1))
                    scorrs = {}
                    for ui, (tt, sc) in enumerate(units):
                        scorrs[(tt, sc)] = psc.tile([128, 512], F32, tag="mmc",
                                                    name=f"scorr{ui % 2}")
                        for pi, (lhs, rhs) in enumerate(
                                ((zl8_t, eh8_t), (zh8_t, el8_t))):
                            for k in range(HT // 2):
                                nc.tensor.matmul(
                                    scorrs[(tt, sc)][:],
                                    lhs[:, 2 * k:2 * k + 2,
                                        tt * 128:(tt + 1) * 128],
                                    rhs[:, 2 * k:2 * k + 2,
                                        sc * 512:(sc + 1) * 512],
                                    start=(pi == 0 and k == 0),
                                    stop=(pi == 1 and k == HT // 2 - 1),
                                    perf_mode=DR)
                    for ui, (tt, sc) in enumerate(units):
                        chunk = sc_tiles[tt][:, sc * 512:(sc + 1) * 512]
                        nc.scalar.activation(
                            out=chunk, in_=spss[(tt, sc)][:],
                            func=mybir.ActivationFunctionType.Copy)
                        nc.vector.scalar_tensor_tensor(
                            out=chunk, in0=scorrs[(tt, sc)][:],
                            scalar=1.0 / (SC_ZL * SC_EH),
                            in1=chunk,
                            op0=mybir.AluOpType.mult,
                            op1=mybir.AluOpType.add)
                        if sc == 1:
                            # both chunks of tt merged: softmax over free dim
                            sc_t = sc_tiles[tt]
                            negmax = stp.tile([128, 1], F32, tag="nm")
                            nc.vector.reduce_max(
                                out=negmax[:], in_=sc_t[:],
                                axis=mybir.AxisListType.X, negate=True)
                            p_t = pp.tile([128, S], F16, tag="p")
                            ssum = stp.tile([128, 1], F32, tag="ss")
                            nc.scalar.activation(
                                out=p_t[:], in_=sc_t[:],
                                func=mybir.ActivationFunctionType.Exp,
                                bias=negmax[:], scale=1.0,
                                accum_out=ssum[:])
                            rsum = stp.tile([128, 1], F32, tag="rs", bufs=8)
                            nc.vector.reciprocal(rsum[:], ssum[:])
                            rsums.append(rsum)
                            if NORM == "now":
                                nc.vector.tensor_scalar_mul(
                                    p_t[:], p_t[:], rsum[:])
                            p_tiles.append(p_t)
                            # transpose p -> pT [s, t] via DMA xbar
                            if TR == "dma":
                                for st in range(ST):
                                    nc.sync.dma_start(
                                        pt_t[:, st, tt * 128:(tt + 1) * 128],
                                        p_t[:, st * 128:(st + 1) * 128],
                                        transpose=True)

                if TR == "pe":
                    # st-major quads: 4 transposes land in one psum bank and
                    # evict as a single [128,512] pt slab, so the PE paces at
                    # the transpose rate instead of the eviction round-trip.
                    for st in range(ST):
                        tps = psc.tile([128, TT, 128], F16, tag="tr", bufs=2)
                        for tt in range(TT):
                            nc.tensor.transpose(
                                tps[:, tt, :],
                                p_tiles[tt][:, st * 128:(st + 1) * 128],
                                ident[:])
                        nc.scalar.activation(
                            out=pt_t[:, st, :], in_=tps[:],
                            func=mybir.ActivationFunctionType.Copy)

                # --- cT = enc_nat.T @ pT -> [k, t] f16 (unnormalized) ---
                ct_t = ctp.tile([128, HT, T], F16, tag="ct")
                for kt in range(HT):
                    cps = psmm.tile([128, T], F32, tag="mm")
                    for st in range(ST):
                        nc.tensor.matmul(
                            cps[:],
                            en_t[:, st, kt * 128:(kt + 1) * 128],
                            pt_t[:, st, :],
                            start=(st == 0), stop=(st == ST - 1))
                    nc.scalar.activation(
                        out=ct_t[:, kt, :], in_=cps[:],
                        func=mybir.ActivationFunctionType.Copy)

                # --- out = tanh(cT.T @ WcT * rsum + qT.T @ WqT + b) ---
                for tt in range(TT):
                    for hc in range(2):
                        # q-part first: gives tail cT evictions extra slack
                        oq = psc.tile([128, 512], F32, tag="mmc")
                        for ht in range(HT):
                            nc.tensor.matmul(
                                oq[:],
                                qh_t[:, ht, tt * 128:(tt + 1) * 128],
                                wo_t[:, HT + ht, hc * 512:(hc + 1) * 512],
                                start=(ht == 0), stop=(ht == HT - 1))
                        oc = psmm.tile([128, 512], F32, tag="mm")
                        for kt in range(HT):
                            nc.tensor.matmul(
                                oc[:],
                                ct_t[:, kt, tt * 128:(tt + 1) * 128],
                                wo_t[:, kt, hc * 512:(hc + 1) * 512],
                                start=(kt == 0), stop=(kt == HT - 1))
                        ost = op.tile([128, 512], F32 if has_bias else F16,
                                      tag="os")
                        if NORM == "defer":
                            ocn = op.tile([128, 512], F32, tag="ocn")
                            nc.vector.tensor_scalar_mul(
                                ocn[:], oc[:], rsums[tt][:])
                            nc.vector.tensor_add(ost[:], ocn[:], oq[:])
                        else:
                            nc.vector.tensor_add(ost[:], oc[:], oq[:])
                        if has_bias:
                            ost16 = op.tile([128, 512], F16, tag="os16")
                            nc.vector.tensor_add(
                                ost[:], ost[:],
                                bias_t[:, hc * 512:(hc + 1) * 512])
                            nc.scalar.activation(
                                out=ost16[:], in_=ost[:],
                                func=mybir.ActivationFunctionType.Tanh)
                            ost = ost16
                        else:
                            nc.scalar.activation(
                                out=ost[:], in_=ost[:],
                                func=mybir.ActivationFunctionType.Tanh)
                        nc.sync.dma_start(
                            out_d[b, tt * 128:(tt + 1) * 128,
                                  hc * 512:(hc + 1) * 512],
                            ost[:])

    nc.compile()
    return nc


def _get_nc(has_bias):
    key = ("nc", has_bias, TR, NORM)
    if key not in _CACHE:
        _CACHE[key] = _build(has_bias)
    return _CACHE[key]


def _split16(x):
    hi = x.astype(np.float16)
    lo = (x - hi.astype(np.float32)).astype(np.float32)
    return hi, lo


def _f8(x, scale):
    return (np.asarray(x, np.float32) * np.float32(scale)).astype(
        ml_dtypes.float8_e4m3)


def kernel(query, encoder_outputs, src_lengths, W_in, W_out, b_out):
    query = np.asarray(query, np.float32)
    enc = np.asarray(encoder_outputs, np.float32)
    W_in = np.asarray(W_in, np.float32)
    W_out = np.asarray(W_out, np.float32)
    b_out = np.asarray(b_out, np.float32)
    has_bias = bool(np.any(b_out))

    # host-side layout prep (transposes + fp16 hi/lo splits)
    qT = np.ascontiguousarray(query.transpose(0, 2, 1))        # [B, H, T]
    qh, ql = _split16(qT)
    encT = np.ascontiguousarray(enc.transpose(1, 2, 0))        # [B, H, S]
    eh, el = _split16(encT)
    en = np.ascontiguousarray(enc.transpose(1, 0, 2)).astype(np.float16)
    whf, wlf = _split16(np.ascontiguousarray(W_in.T))          # [H(h), H(i)]
    wo = np.ascontiguousarray(W_out.T).astype(np.float16)      # [2H, H]

    common = {"wh": whf, "wo": wo,
              "wh8": _f8(whf.astype(np.float32), SC_WH),
              "wl8": _f8(wlf, SC_WL)}
    if has_bias:
        common["bias"] = np.ascontiguousarray(
            np.broadcast_to(b_out[None, :], (128, H)), np.float32)

    in_maps = []
    for c in range(NCORES):
        sl = slice(c * BL, (c + 1) * BL)
        m = {
            "qh": np.ascontiguousarray(qh[sl]),
            "eh": np.ascontiguousarray(eh[sl]),
            "en": np.ascontiguousarray(en[sl]),
            "qh8": _f8(qh[sl].astype(np.float32), SC_QH),
            "ql8": _f8(ql[sl], SC_QL),
            "eh8": _f8(eh[sl].astype(np.float32), SC_EH),
            "el8": _f8(el[sl], SC_EL),
            **common,
        }
        in_maps.append(m)

    nc = _get_nc(has_bias)
    trace = bool(int(os.environ.get("KERNEL_TRACE", "0")))
    res = run_bass_kernel_spmd(nc, in_maps, core_ids=list(range(NCORES)),
                               trace=trace)
    if trace:
        _CACHE["last_exec_time_ns"] = res.exec_time_ns
        _CACHE["last_results"] = res
    out = np.concatenate([r["out"] for r in res.results], axis=0)
    return out.astype(np.float32)


# revision 20
# speedup vs baseline: 1.3261x; 1.0013x over previous
"""Trainium2 Bass kernel for nn_Attention_80779744903968.

Reference computation (B=32, T=512, S=1024, H=1024):
    z      = q @ W_in.T                  [B,T,H]
    scores = z @ enc_b.T                 [B,T,S]   (enc input is [S,B,H])
    p      = softmax(scores, axis=-1)    (the scores==0 -> -inf fill is a
                                          numerical no-op: row maxes are ~120,
                                          exp(0-max) == 0 in fp32)
    c      = p @ enc_b                   [B,T,H]
    out    = tanh([c, q] @ W_out.T + b)  [B,T,H]

Sharding: data-parallel over B across 8 cores (4 batches per core).
W_in / W_out replicated.

Precision strategy (PE matmuls):
  - z and scores need near-fp32 logits: the softmax is near-one-hot
    (logit std ~37) with near-tied rows (min top-2 gap ~2e-4), so every
    one of the four correction products matters for the absmax.
  - Both are computed as an fp16 hi/lo split: x*y ~= xh*yh (fp16 main
    pass, fp32 PSUM accumulation) + (xh*yl + xl*yh) correction passes in
    fp8(e4m3) with perf_mode=DoubleRow at 2 contraction-tiles per
    matmul; correction operands are pre-scaled by powers of 2 so both
    corr products share one PSUM scale.
  - downstream (p, enc, c, q, W_out) runs in plain fp16: p is in [0,1]
    and c/out magnitudes are O(1), so fp16's 2^-11 relative error is
    plenty.

Schedule notes (perf):
  - z and scores phases run quad-grouped: 32 fp16 main matmuls, then 32
    fp8-DR correction matmuls (pass-major).  fp16<->fp8 PE mode switches
    cost ~0.4us each, so fewer/longer runs win.
  - hi/lo merge is a single scalar_tensor_tensor reading BOTH psum banks
    (main + corr), freeing them in one DVE op.
  - softmax: exp runs on the Scalar engine with accum_out producing the
    row sum for free; p is NOT normalized -- the 1/sum scale is applied
    at the out-projection eviction (out rows have t on partitions, so
    rsum is a per-partition scalar there).  The out projection therefore
    accumulates its c-part and q-part in separate psum banks.
  - p is transposed by the DMA xbar (SBUF->SBUF, fp16) instead of the
    PE; the c matmul consumes the transposed tile directly.

All input transposes (q -> [H,T], enc -> [H,S] per batch) are done on
the host so every device-side input DMA is a contiguous natural-layout
load.  Output is written fp16 and cast to fp32 on the host.
"""
import os
import sys

import numpy as np

sys.path.insert(0, "/opt/trn_rl_repo")

import ml_dtypes  # noqa: E402

import concourse.bass as bass  # noqa: E402
import concourse.tile as tile  # noqa: E402
from concourse import bacc, mybir  # noqa: E402
from concourse.bass_utils import run_bass_kernel_spmd  # noqa: E402
from concourse.masks import make_identity  # noqa: E402

B, T, S, H = 32, 512, 1024, 1024
NCORES = 8
BL = B // NCORES  # batches per core
HT = H // 128     # h/i/k tiles per 1024
TT = T // 128     # t tiles
ST = S // 128     # s tiles
F16 = mybir.dt.float16
F32 = mybir.dt.float32
F8 = mybir.dt.float8e4
DR = mybir.MatmulPerfMode.DoubleRow

TR = os.environ.get("KERNEL_TR", "dma")      # p-transpose: dma xbar | pe
NORM = os.environ.get("KERNEL_NORM", "defer")  # softmax norm: defer | now

# power-of-2 scales for fp8 correction operands (products must share scale)
SC_WH, SC_WL, SC_QH, SC_QL = 2.0**4, 2.0**15, 1.0, 2.0**11   # z corr: 2^15
SC_ZH, SC_ZL, SC_EH, SC_EL = 1.0, 2.0**12, 1.0, 2.0**12      # s corr: 2^12

_CACHE = {}


def _build(has_bias):
    nc = bacc.Bacc("TRN2", target_bir_lowering=False, debug=False,
                   num_devices=NCORES)

    def din(name, shape, dt=F16):
        return nc.dram_tensor(name, shape, dt, kind="ExternalInput").ap()

    qh_d = din("qh", [BL, H, T])
    eh_d = din("eh", [BL, H, S])
    en_d = din("en", [BL, S, H])
    wh_d = din("wh", [H, H])
    wo_d = din("wo", [2 * H, H])
    if has_bias:
        bias_d = din("bias", [128, H], F32)
    ql8_d = din("ql8", [BL, H, T], F8)
    el8_d = din("el8", [BL, H, S], F8)
    qh8_d = din("qh8", [BL, H, T], F8)
    eh8_d = din("eh8", [BL, H, S], F8)
    wh8_d = din("wh8", [H, H], F8)
    wl8_d = din("wl8", [H, H], F8)
    out_d = nc.dram_tensor("out", [BL, T, H], F16, kind="ExternalOutput").ap()

    with tile.TileContext(nc) as tc:
        with (
            tc.tile_pool(name="weights", bufs=1) as wp,
            tc.tile_pool(name="qin", bufs=2) as qp,
            tc.tile_pool(name="ein", bufs=1) as ep,
            tc.tile_pool(name="enin", bufs=1) as enp,
            tc.tile_pool(name="zbuf", bufs=1) as zp,
            tc.tile_pool(name="scratch", bufs=2) as scrp,
            tc.tile_pool(name="scores", bufs=2) as scp,
            tc.tile_pool(name="pbuf", bufs=4) as pp,
            tc.tile_pool(name="ptbuf", bufs=1) as ptp,
            tc.tile_pool(name="ctbuf", bufs=1) as ctp,
            tc.tile_pool(name="ostage", bufs=3) as op,
            tc.tile_pool(name="stats", bufs=4) as stp,
            tc.tile_pool(name="psmm", bufs=4, space="PSUM") as psmm,
            tc.tile_pool(name="psc", bufs=2, space="PSUM") as psc,
        ):
            # --- resident weights / constants ---
            # queue order matters at startup: chunk wh/qh0 by ht so the first
            # z matmul waits on ~384KB, not 3MB (DMA sems are per-transfer);
            # z corr pass 1 needs wl8+qh8 so those precede wh8/ql8.
            wh_t = wp.tile([128, HT, H], F16)
            wh_r = wh_d.rearrange("(ht p) i -> p ht i", p=128)
            qh_first = qp.tile([128, HT, T], F16, tag="qh")
            qh_r = qh_d[0].rearrange("(ht p) t -> p ht t", p=128)
            for ht in range(HT):
                nc.sync.dma_start(wh_t[:, ht, :], wh_r[:, ht, :])
                nc.sync.dma_start(qh_first[:, ht, :], qh_r[:, ht, :])
            wl8_t = wp.tile([128, HT, H], F8)
            nc.sync.dma_start(
                wl8_t[:], wl8_d.rearrange("(ht p) i -> p ht i", p=128))
            qh8_first = qp.tile([128, HT, T], F8, tag="qh8", bufs=1)
            nc.sync.dma_start(
                qh8_first[:], qh8_d[0].rearrange("(ht p) t -> p ht t", p=128))
            wh8_t = wp.tile([128, HT, H], F8)
            nc.sync.dma_start(
                wh8_t[:], wh8_d.rearrange("(ht p) i -> p ht i", p=128))
            ql8_first = qp.tile([128, HT, T], F8, tag="ql8", bufs=1)
            nc.sync.dma_start(
                ql8_first[:], ql8_d[0].rearrange("(ht p) t -> p ht t", p=128))
            if TR == "pe":
                ident = wp.tile([128, 128], F16)
                make_identity(nc, ident[:])
            wo_t = None
            bias_t = None

            for b in range(BL):
                if b == 0:
                    qh_t, qh8_t, ql8_t = qh_first, qh8_first, ql8_first
                else:
                    qh_t = qp.tile([128, HT, T], F16, tag="qh")
                    nc.sync.dma_start(
                        qh_t[:], qh_d[b].rearrange("(ht p) t -> p ht t", p=128))
                    qh8_t = qp.tile([128, HT, T], F8, tag="qh8", bufs=1)
                    nc.sync.dma_start(
                        qh8_t[:], qh8_d[b].rearrange("(ht p) t -> p ht t", p=128))
                    ql8_t = qp.tile([128, HT, T], F8, tag="ql8", bufs=1)
                    nc.sync.dma_start(
                        ql8_t[:], ql8_d[b].rearrange("(ht p) t -> p ht t", p=128))
                eh_t = ep.tile([128, HT, S], F16, tag="eh")
                nc.sync.dma_start(
                    eh_t[:], eh_d[b].rearrange("(it p) s -> p it s", p=128))
                eh8_t = ep.tile([128, HT, S], F8, tag="eh8")
                nc.sync.dma_start(
                    eh8_t[:], eh8_d[b].rearrange("(it p) s -> p it s", p=128))
                el8_t = ep.tile([128, HT, S], F8, tag="el8")
                nc.sync.dma_start(
                    el8_t[:], el8_d[b].rearrange("(it p) s -> p it s", p=128))
                en_t = enp.tile([128, ST, H], F16, tag="en")
                nc.sync.dma_start(
                    en_t[:], en_d[b].rearrange("(st p) k -> p st k", p=128))
                if wo_t is None:
                    # deferred: b0's z/scores inputs take queue priority
                    wo_t = wp.tile([128, 2 * HT, H], F16)
                    nc.sync.dma_start(
                        wo_t[:], wo_d.rearrange("(kt p) h -> p kt h", p=128))
                    if has_bias:
                        bias_t = wp.tile([128, H], F32)
                        nc.sync.dma_start(bias_t[:], bias_d)

                # --- zT = W_inT.T @ qT (hi/lo) -> zh (f16) + fp8 lo forms ---
                zh_t = zp.tile([128, HT, T], F16, tag="zh")
                zh8_t = zp.tile([128, HT, T], F8, tag="zh8")
                zl8_t = zp.tile([128, HT, T], F8, tag="zl8")

                # phase-wide mode runs: one 64-matmul fp16 run, then one
                # 64-matmul fp8-DR run (a fp16<->fp8 PE mode switch costs
                # ~0.6us, so fewer/longer runs win).  Main psums park in an
                # SBUF f32 scratch ring until their correction merges.
                zmains = {}
                for quad in range(HT // 4):
                    its = range(quad * 4, quad * 4 + 4)
                    zpss = {it: psmm.tile([128, T], F32, tag="mm",
                                          name=f"zps{it % 4}")
                            for it in its}
                    # ht-major so b0 overlaps the chunked wh/qh arrival
                    for ht in range(HT):
                        for it in its:
                            nc.tensor.matmul(
                                zpss[it][:],
                                wh_t[:, ht, it * 128:(it + 1) * 128],
                                qh_t[:, ht, :],
                                start=(ht == 0), stop=(ht == HT - 1))
                    for it in its:
                        zmains[it] = scrp.tile([128, T], F32, tag="pmain",
                                               bufs=8, name=f"zmain{it}")
                        nc.vector.tensor_copy(zmains[it][:], zpss[it][:])
                for it in range(HT):
                    zcorr = psc.tile([128, T], F32, tag="mmc",
                                     name=f"zcorr{it % 2}")
                    for pi, (lhs, rhs) in enumerate(
                            ((wl8_t, qh8_t), (wh8_t, ql8_t))):
                        for k in range(HT // 2):
                            nc.tensor.matmul(
                                zcorr[:],
                                lhs[:, 2 * k:2 * k + 2,
                                    it * 128:(it + 1) * 128],
                                rhs[:, 2 * k:2 * k + 2, :],
                                start=(pi == 0 and k == 0),
                                stop=(pi == 1 and k == HT // 2 - 1),
                                perf_mode=DR)
                    # merge: comb = zmain + zcorr * 2^-19, split into f16 +
                    # scaled f8 lo.  Casts run on the Scalar engine so DVE
                    # (STT + sub = 1.4us/it) keeps up with the PE's corr
                    # pace (2.0us/it) and psum banks recycle promptly.
                    comb = scrp.tile([128, T], F32, tag="comb")
                    nc.vector.scalar_tensor_tensor(
                        out=comb[:], in0=zcorr[:],
                        scalar=1.0 / (SC_WH * SC_QL), in1=zmains[it][:],
                        op0=mybir.AluOpType.mult, op1=mybir.AluOpType.add)
                    nc.scalar.activation(
                        out=zh_t[:, it, :], in_=comb[:],
                        func=mybir.ActivationFunctionType.Copy)
                    zl_tmp = scrp.tile([128, T], F16, tag="zltmp", bufs=1)
                    nc.vector.tensor_sub(zl_tmp[:], comb[:],
                                         zh_t[:, it, :])
                    nc.scalar.activation(
                        out=zl8_t[:, it, :], in_=zl_tmp[:],
                        func=mybir.ActivationFunctionType.Copy,
                        scale=SC_ZL)
                    nc.scalar.activation(
                        out=zh8_t[:, it, :], in_=zh_t[:, it, :],
                        func=mybir.ActivationFunctionType.Copy)

                # --- scores + softmax -> p (f16, unnormalized) ---
                # units u = (tt, sc): quad = 2 tt x 2 sc so softmax for a tt
                # can start right after its quad merges.
                p_tiles = []
                rsums = []
                sc_tiles = {}
                pt_t = ptp.tile([128, ST, T], F16, tag="pt")
                units = [(tt, sc) for tt in range(TT) for sc in range(2)]
                smains = {}
                for quad in range(2):
                    qunits = units[quad * 4:quad * 4 + 4]
                    spss = {}
                    for (tt, sc) in qunits:
                        sps = psmm.tile([128, 512], F32, tag="mm",
                                        name=f"sps{tt % 2}_{sc}")
                        spss[(tt, sc)] = sps
                        for it in range(HT):
                            nc.tensor.matmul(
                                sps[:],
                                zh_t[:, it, tt * 128:(tt + 1) * 128],
                                eh_t[:, it, sc * 512:(sc + 1) * 512],
                                start=(it == 0), stop=(it == HT - 1))
                    for ui, (tt, sc) in enumerate(qunits):
                        smains[(tt, sc)] = scrp.tile(
                            [128, 512], F32, tag="pmain", bufs=8,
                            name=f"smain{ui}")
                        nc.vector.tensor_copy(smains[(tt, sc)][:],
                                              spss[(tt, sc)][:])
                for ui, (tt, sc) in enumerate(units):
                    if sc == 0:
                        sc_tiles[tt] = scp.tile([128, S], F32, tag="sc",
                                                name=f"sc{tt % 2}")
                    scorr = psc.tile([128, 512], F32, tag="mmc",
                                     name=f"scorr{ui % 2}")
                    for pi, (lhs, rhs) in enumerate(
                            ((zl8_t, eh8_t), (zh8_t, el8_t))):
                        for k in range(HT // 2):
                            nc.tensor.matmul(
                                scorr[:],
                                lhs[:, 2 * k:2 * k + 2,
                                    tt * 128:(tt + 1) * 128],
                                rhs[:, 2 * k:2 * k + 2,
                                    sc * 512:(sc + 1) * 512],
                                start=(pi == 0 and k == 0),
                                stop=(pi == 1 and k == HT // 2 - 1),
                                perf_mode=DR)
                    chunk = sc_tiles[tt][:, sc * 512:(sc + 1) * 512]
                    nc.vector.scalar_tensor_tensor(
                        out=chunk, in0=scorr[:],
                        scalar=1.0 / (SC_ZL * SC_EH),
                        in1=smains[(tt, sc)][:],
                        op0=mybir.AluOpType.mult,
                        op1=mybir.AluOpType.add)
                    if True:
                        if sc == 1:
                            # both chunks of tt merged: softmax over free dim
                            sc_t = sc_tiles[tt]
                            negmax = stp.tile([128, 1], F32, tag="nm")
                            nc.vector.reduce_max(
                                out=negmax[:], in_=sc_t[:],
                                axis=mybir.AxisListType.X, negate=True)
                            p_t = pp.tile([128, S], F16, tag="p")
                            ssum = stp.tile([128, 1], F32, tag="ss")
                            nc.scalar.activation(
                                out=p_t[:], in_=sc_t[:],
                                func=mybir.ActivationFunctionType.Exp,
                                bias=negmax[:], scale=1.0,
                                accum_out=ssum[:])
                            rsum = stp.tile([128, 1], F32, tag="rs", bufs=8)
                            nc.vector.reciprocal(rsum[:], ssum[:])
                            rsums.append(rsum)
                            if NORM == "now":
                                nc.vector.tensor_scalar_mul(
                                    p_t[:], p_t[:], rsum[:])
                            p_tiles.append(p_t)
                            # transpose p -> pT [s, t] via DMA xbar
                            if TR == "dma":
                                for st in range(ST):
                                    nc.sync.dma_start(
                                        pt_t[:, st, tt * 128:(tt + 1) * 128],
                                        p_t[:, st * 128:(st + 1) * 128],
                                        transpose=True)

                if TR == "pe":
                    # st-major quads: 4 transposes land in one psum bank and
                    # evict as a single [128,512] pt slab, so the PE paces at
                    # the transpose rate instead of the eviction round-trip.
                    for st in range(ST):
                        tps = psc.tile([128, TT, 128], F16, tag="tr", bufs=2)
                        for tt in range(TT):
                            nc.tensor.transpose(
                                tps[:, tt, :],
                                p_tiles[tt][:, st * 128:(st + 1) * 128],
                                ident[:])
                        nc.scalar.activation(
                            out=pt_t[:, st, :], in_=tps[:],
                            func=mybir.ActivationFunctionType.Copy)

                # --- cT = enc_nat.T @ pT -> [k, t] f16 (unnormalized) ---
                ct_t = ctp.tile([128, HT, T], F16, tag="ct")
                for kt in range(HT):
                    cps = psmm.tile([128, T], F32, tag="mm")
                    for st in range(ST):
                        nc.tensor.matmul(
                            cps[:],
                            en_t[:, st, kt * 128:(kt + 1) * 128],
                            pt_t[:, st, :],
                            start=(st == 0), stop=(st == ST - 1))
                    nc.scalar.activation(
                        out=ct_t[:, kt, :], in_=cps[:],
                        func=mybir.ActivationFunctionType.Copy)

                # --- out = tanh(cT.T @ WcT * rsum + qT.T @ WqT + b) ---
                for tt in range(TT):
                    for hc in range(2):
                        # q-part first: gives tail cT evictions extra slack
                        oq = psc.tile([128, 512], F32, tag="mmc")
                        for ht in range(HT):
                            nc.tensor.matmul(
                                oq[:],
                                qh_t[:, ht, tt * 128:(tt + 1) * 128],
                                wo_t[:, HT + ht, hc * 512:(hc + 1) * 512],
                                start=(ht == 0), stop=(ht == HT - 1))
                        oc = psmm.tile([128, 512], F32, tag="mm")
                        for kt in range(HT):
                            nc.tensor.matmul(
                                oc[:],
                                ct_t[:, kt, tt * 128:(tt + 1) * 128],
                                wo_t[:, kt, hc * 512:(hc + 1) * 512],
                                start=(kt == 0), stop=(kt == HT - 1))
                        ost = op.tile([128, 512], F32 if has_bias else F16,
                                      tag="os", bufs=2)
                        if NORM == "defer":
                            ocn = scrp.tile([128, 512], F32, tag="comb")
                            nc.vector.tensor_scalar_mul(
                                ocn[:], oc[:], rsums[tt][:])
                            nc.vector.tensor_add(ost[:], ocn[:], oq[:])
                        else:
                            nc.vector.tensor_add(ost[:], oc[:], oq[:])
                        if has_bias:
                            ost16 = op.tile([128, 512], F16, tag="os16")
                            nc.vector.tensor_add(
                                ost[:], ost[:],
                                bias_t[:, hc * 512:(hc + 1) * 512])
                            nc.scalar.activation(
                                out=ost16[:], in_=ost[:],
                                func=mybir.ActivationFunctionType.Tanh)
                            ost = ost16
                        else:
                            nc.scalar.activation(
                                out=ost[:], in_=ost[:],
                                func=mybir.ActivationFunctionType.Tanh)
                        nc.sync.dma_start(
                            out_d[b, tt * 128:(tt + 1) * 128,
                                  hc * 512:(hc + 1) * 512],
                            ost[:])

    nc.compile()
    return nc


def _get_nc(has_bias):
    key = ("nc", has_bias, TR, NORM)
    if key not in _CACHE:
        _CACHE[key] = _build(has_bias)
    return _CACHE[key]


def _split16(x):
    hi = x.astype(np.float16)
    lo = (x - hi.astype(np.float32)).astype(np.float32)
    return hi, lo


def _f8(x, scale):
    return (np.asarray(x, np.float32) * np.float32(scale)).astype(
        ml_dtypes.float8_e4m3)


def kernel(query, encoder_outputs, src_lengths, W_in, W_out, b_out):
    query = np.asarray(query, np.float32)
    enc = np.asarray(encoder_outputs, np.float32)
    W_in = np.asarray(W_in, np.float32)
    W_out = np.asarray(W_out, np.float32)
    b_out = np.asarray(b_out, np.float32)
    has_bias = bool(np.any(b_out))

    # host-side layout prep (transposes + fp16 hi/lo splits)
    qT = np.ascontiguousarray(query.transpose(0, 2, 1))        # [B, H, T]
    qh, ql = _split16(qT)
    encT = np.ascontiguousarray(enc.transpose(1, 2, 0))        # [B, H, S]
    eh, el = _split16(encT)
    en = np.ascontiguousarray(enc.transpose(1, 0, 2)).astype(np.float16)
    whf, wlf = _split16(np.ascontiguousarray(W_in.T))          # [H(h), H(i)]
    wo = np.ascontiguousarray(W_out.T).astype(np.float16)      # [2H, H]

    common = {"wh": whf, "wo": wo,
              "wh8": _f8(whf.astype(np.float32), SC_WH),
              "wl8": _f8(wlf, SC_WL)}
    if has_bias:
        common["bias"] = np.ascontiguousarray(
            np.broadcast_to(b_out[None, :], (128, H)), np.float32)

    in_maps = []
    for c in range(NCORES):
        sl = slice(c * BL, (c + 1) * BL)
        m = {
            "qh": np.ascontiguousarray(qh[sl]),
            "eh": np.ascontiguousarray(eh[sl]),
            "en": np.ascontiguousarray(en[sl]),
            "qh8": _f8(qh[sl].astype(np.float32), SC_QH),
            "ql8": _f8(ql[sl], SC_QL),
            "eh8": _f8(eh[sl].astype(np.float32), SC_EH),
            "el8": _f8(el[sl], SC_EL),
            **common,
        }
        in_maps.append(m)

    nc = _get_nc(has_bias)
    trace = bool(int(os.environ.get("KERNEL_TRACE", "0")))
    res = run_bass_kernel_spmd(nc, in_maps, core_ids=list(range(NCORES)),
                               trace=trace)
    if trace:
        _CACHE["last_exec_time_ns"] = res.exec_time_ns
        _CACHE["last_results"] = res
    out = np.concatenate([r["out"] for r in res.results], axis=0)
    return out.astype(np.float32)


# revision 23
# speedup vs baseline: 1.3336x; 1.0057x over previous
"""Trainium2 Bass kernel for nn_Attention_80779744903968.

Reference computation (B=32, T=512, S=1024, H=1024):
    z      = q @ W_in.T                  [B,T,H]
    scores = z @ enc_b.T                 [B,T,S]   (enc input is [S,B,H])
    p      = softmax(scores, axis=-1)    (the scores==0 -> -inf fill is a
                                          numerical no-op: row maxes are ~120,
                                          exp(0-max) == 0 in fp32)
    c      = p @ enc_b                   [B,T,H]
    out    = tanh([c, q] @ W_out.T + b)  [B,T,H]

Sharding: data-parallel over B across 8 cores (4 batches per core).
W_in / W_out replicated.

Precision strategy (PE matmuls):
  - z and scores need near-fp32 logits: the softmax is near-one-hot
    (logit std ~37) with near-tied rows (min top-2 gap ~2e-4), so every
    one of the four correction products matters for the absmax.
  - Both are computed as an fp16 hi/lo split: x*y ~= xh*yh (fp16 main
    pass, fp32 PSUM accumulation) + (xh*yl + xl*yh) correction passes in
    fp8(e4m3) with perf_mode=DoubleRow at 2 contraction-tiles per
    matmul; correction operands are pre-scaled by powers of 2 so both
    corr products share one PSUM scale.
  - downstream (p, enc, c, q, W_out) runs in plain fp16: p is in [0,1]
    and c/out magnitudes are O(1), so fp16's 2^-11 relative error is
    plenty.

Schedule notes (perf):
  - z and scores phases run quad-grouped: 32 fp16 main matmuls, then 32
    fp8-DR correction matmuls (pass-major).  fp16<->fp8 PE mode switches
    cost ~0.4us each, so fewer/longer runs win.
  - hi/lo merge is a single scalar_tensor_tensor reading BOTH psum banks
    (main + corr), freeing them in one DVE op.
  - softmax: exp runs on the Scalar engine with accum_out producing the
    row sum for free; p is NOT normalized -- the 1/sum scale is applied
    at the out-projection eviction (out rows have t on partitions, so
    rsum is a per-partition scalar there).  The out projection therefore
    accumulates its c-part and q-part in separate psum banks.
  - p is transposed by the DMA xbar (SBUF->SBUF, fp16) instead of the
    PE; the c matmul consumes the transposed tile directly.

All input transposes (q -> [H,T], enc -> [H,S] per batch) are done on
the host so every device-side input DMA is a contiguous natural-layout
load.  Output is written fp16 and cast to fp32 on the host.
"""
import os
import sys

import numpy as np

sys.path.insert(0, "/opt/trn_rl_repo")

import ml_dtypes  # noqa: E402

import concourse.bass as bass  # noqa: E402
import concourse.tile as tile  # noqa: E402
from concourse import bacc, mybir  # noqa: E402
from concourse.bass_utils import run_bass_kernel_spmd  # noqa: E402
from concourse.masks import make_identity  # noqa: E402

B, T, S, H = 32, 512, 1024, 1024
NCORES = 8
BL = B // NCORES  # batches per core
HT = H // 128     # h/i/k tiles per 1024
TT = T // 128     # t tiles
ST = S // 128     # s tiles
F16 = mybir.dt.float16
F32 = mybir.dt.float32
F8 = mybir.dt.float8e4
DR = mybir.MatmulPerfMode.DoubleRow

TR = os.environ.get("KERNEL_TR", "dma")      # p-transpose: dma xbar | pe
NORM = os.environ.get("KERNEL_NORM", "defer")  # softmax norm: defer | now

# power-of-2 scales for fp8 correction operands (products must share scale)
SC_WH, SC_WL, SC_QH, SC_QL = 2.0**4, 2.0**15, 1.0, 2.0**11   # z corr: 2^15
SC_ZH, SC_ZL, SC_EH, SC_EL = 1.0, 2.0**12, 1.0, 2.0**12      # s corr: 2^12

_CACHE = {}


def _build(has_bias):
    nc = bacc.Bacc("TRN2", target_bir_lowering=False, debug=False,
                   num_devices=NCORES)

    def din(name, shape, dt=F16):
        return nc.dram_tensor(name, shape, dt, kind="ExternalInput").ap()

    qh_d = din("qh", [BL, H, T])
    eh_d = din("eh", [BL, H, S])
    en_d = din("en", [BL, S, H])
    wh_d = din("wh", [H, H])
    wo_d = din("wo", [2 * H, H])
    if has_bias:
        bias_d = din("bias", [128, H], F32)
    ql8_d = din("ql8", [BL, H, T], F8)
    el8_d = din("el8", [BL, H, S], F8)
    qh8_d = din("qh8", [BL, H, T], F8)
    eh8_d = din("eh8", [BL, H, S], F8)
    wh8_d = din("wh8", [H, H], F8)
    wl8_d = din("wl8", [H, H], F8)
    out_d = nc.dram_tensor("out", [BL, T, H], F16, kind="ExternalOutput").ap()

    with tile.TileContext(nc) as tc:
        with (
            tc.tile_pool(name="weights", bufs=1) as wp,
            tc.tile_pool(name="qin", bufs=2) as qp,
            tc.tile_pool(name="ein", bufs=1) as ep,
            tc.tile_pool(name="enin", bufs=1) as enp,
            tc.tile_pool(name="zbuf", bufs=1) as zp,
            tc.tile_pool(name="scratch", bufs=2) as scrp,
            tc.tile_pool(name="scores", bufs=2) as scp,
            tc.tile_pool(name="pbuf", bufs=4) as pp,
            tc.tile_pool(name="ptbuf", bufs=1) as ptp,
            tc.tile_pool(name="ctbuf", bufs=1) as ctp,
            tc.tile_pool(name="ostage", bufs=3) as op,
            tc.tile_pool(name="stats", bufs=4) as stp,
            tc.tile_pool(name="psmm", bufs=4, space="PSUM") as psmm,
            tc.tile_pool(name="psc", bufs=2, space="PSUM") as psc,
        ):
            # --- resident weights / constants ---
            # queue order matters at startup: chunk wh/qh0 by ht so the first
            # z matmul waits on ~384KB, not 3MB (DMA sems are per-transfer);
            # z corr pass 1 needs wl8+qh8 so those precede wh8/ql8.
            wh_t = wp.tile([128, HT, H], F16)
            wh_r = wh_d.rearrange("(ht p) i -> p ht i", p=128)
            qh_first = qp.tile([128, HT, T], F16, tag="qh")
            qh_r = qh_d[0].rearrange("(ht p) t -> p ht t", p=128)
            for ht in range(HT):
                nc.sync.dma_start(wh_t[:, ht, :], wh_r[:, ht, :])
                nc.sync.dma_start(qh_first[:, ht, :], qh_r[:, ht, :])
            wl8_t = wp.tile([128, HT, H], F8)
            nc.sync.dma_start(
                wl8_t[:], wl8_d.rearrange("(ht p) i -> p ht i", p=128))
            qh8_first = qp.tile([128, HT, T], F8, tag="qh8", bufs=1)
            nc.sync.dma_start(
                qh8_first[:], qh8_d[0].rearrange("(ht p) t -> p ht t", p=128))
            wh8_t = wp.tile([128, HT, H], F8)
            nc.sync.dma_start(
                wh8_t[:], wh8_d.rearrange("(ht p) i -> p ht i", p=128))
            ql8_first = qp.tile([128, HT, T], F8, tag="ql8", bufs=1)
            nc.sync.dma_start(
                ql8_first[:], ql8_d[0].rearrange("(ht p) t -> p ht t", p=128))
            if TR == "pe":
                ident = wp.tile([128, 128], F16)
                make_identity(nc, ident[:])
            wo_t = None
            bias_t = None

            for b in range(BL):
                if b == 0:
                    qh_t, qh8_t, ql8_t = qh_first, qh8_first, ql8_first
                else:
                    qh_t = qp.tile([128, HT, T], F16, tag="qh")
                    nc.sync.dma_start(
                        qh_t[:], qh_d[b].rearrange("(ht p) t -> p ht t", p=128))
                    qh8_t = qp.tile([128, HT, T], F8, tag="qh8", bufs=1)
                    nc.sync.dma_start(
                        qh8_t[:], qh8_d[b].rearrange("(ht p) t -> p ht t", p=128))
                    ql8_t = qp.tile([128, HT, T], F8, tag="ql8", bufs=1)
                    nc.sync.dma_start(
                        ql8_t[:], ql8_d[b].rearrange("(ht p) t -> p ht t", p=128))
                eh_t = ep.tile([128, HT, S], F16, tag="eh", bufs=2)
                nc.sync.dma_start(
                    eh_t[:], eh_d[b].rearrange("(it p) s -> p it s", p=128))
                eh8_t = ep.tile([128, HT, S], F8, tag="eh8")
                nc.sync.dma_start(
                    eh8_t[:], eh8_d[b].rearrange("(it p) s -> p it s", p=128))
                el8_t = ep.tile([128, HT, S], F8, tag="el8")
                nc.sync.dma_start(
                    el8_t[:], el8_d[b].rearrange("(it p) s -> p it s", p=128))
                en_t = enp.tile([128, ST, H], F16, tag="en")
                nc.sync.dma_start(
                    en_t[:], en_d[b].rearrange("(st p) k -> p st k", p=128))
                if wo_t is None:
                    # deferred: b0's z/scores inputs take queue priority
                    wo_t = wp.tile([128, 2 * HT, H], F16)
                    nc.sync.dma_start(
                        wo_t[:], wo_d.rearrange("(kt p) h -> p kt h", p=128))
                    if has_bias:
                        bias_t = wp.tile([128, H], F32)
                        nc.sync.dma_start(bias_t[:], bias_d)

                # --- zT = W_inT.T @ qT (hi/lo) -> zh (f16) + fp8 lo forms ---
                zh_t = zp.tile([128, HT, T], F16, tag="zh")
                zh8_t = zp.tile([128, HT, T], F8, tag="zh8")
                zl8_t = zp.tile([128, HT, T], F8, tag="zl8")

                # quad-grouped: 32 fp16 main matmuls, then 32 fp8-DR corr
                # matmuls (a fp16<->fp8 PE mode switch costs ~0.6us, so
                # fewer/longer runs win).
                for quad in range(HT // 4):
                    its = range(quad * 4, quad * 4 + 4)
                    zpss = {it: psmm.tile([128, T], F32, tag="mm",
                                          name=f"zps{it % 4}")
                            for it in its}
                    # ht-major so b0 overlaps the chunked wh/qh arrival
                    for ht in range(HT):
                        for it in its:
                            nc.tensor.matmul(
                                zpss[it][:],
                                wh_t[:, ht, it * 128:(it + 1) * 128],
                                qh_t[:, ht, :],
                                start=(ht == 0), stop=(ht == HT - 1))
                    zcorrs = {it: psc.tile([128, T], F32, tag="mmc",
                                           name=f"zcorr{it % 2}")
                              for it in its}
                    for it in its:
                        for pi, (lhs, rhs) in enumerate(
                                ((wl8_t, qh8_t), (wh8_t, ql8_t))):
                            for k in range(HT // 2):
                                nc.tensor.matmul(
                                    zcorrs[it][:],
                                    lhs[:, 2 * k:2 * k + 2,
                                        it * 128:(it + 1) * 128],
                                    rhs[:, 2 * k:2 * k + 2, :],
                                    start=(pi == 0 and k == 0),
                                    stop=(pi == 1 and k == HT // 2 - 1),
                                    perf_mode=DR)
                    # merge: comb = zps + zcorr * 2^-19, split into f16 +
                    # scaled f8 lo.  Casts run on the Scalar engine so DVE
                    # (copy + STT + sub = 1.75us/it) keeps up with the PE's
                    # corr pace (2.0us/it) and psum banks recycle promptly.
                    for it in its:
                        comb = scrp.tile([128, T], F32, tag="comb")
                        nc.vector.tensor_copy(comb[:], zpss[it][:])
                        nc.vector.scalar_tensor_tensor(
                            out=comb[:], in0=zcorrs[it][:],
                            scalar=1.0 / (SC_WH * SC_QL), in1=comb[:],
                            op0=mybir.AluOpType.mult, op1=mybir.AluOpType.add)
                        nc.scalar.activation(
                            out=zh_t[:, it, :], in_=comb[:],
                            func=mybir.ActivationFunctionType.Copy)
                        zl_tmp = scrp.tile([128, T], F16, tag="zltmp",
                                           bufs=1)
                        nc.vector.tensor_sub(zl_tmp[:], comb[:],
                                             zh_t[:, it, :])
                        nc.scalar.activation(
                            out=zl8_t[:, it, :], in_=zl_tmp[:],
                            func=mybir.ActivationFunctionType.Copy,
                            scale=SC_ZL)
                        nc.scalar.activation(
                            out=zh8_t[:, it, :], in_=zh_t[:, it, :],
                            func=mybir.ActivationFunctionType.Copy)

                # --- scores + softmax -> p (f16, unnormalized) ---
                # units u = (tt, sc): quad = 2 tt x 2 sc so softmax for a tt
                # can start right after its quad merges.
                p_tiles = []
                rsums = []
                sc_tiles = {}
                pt_t = ptp.tile([128, ST, T], F16, tag="pt")
                for quad in range(2):
                    units = [(quad * 2 + dt, sc) for dt in range(2)
                             for sc in range(2)]
                    spss = {}
                    for (tt, sc) in units:
                        if sc == 0:
                            sc_tiles[tt] = scp.tile([128, S], F32, tag="sc",
                                                    name=f"sc{tt % 2}")
                        sps = psmm.tile([128, 512], F32, tag="mm",
                                        name=f"sps{tt % 2}_{sc}")
                        spss[(tt, sc)] = sps
                        for it in range(HT):
                            nc.tensor.matmul(
                                sps[:],
                                zh_t[:, it, tt * 128:(tt + 1) * 128],
                                eh_t[:, it, sc * 512:(sc + 1) * 512],
                                start=(it == 0), stop=(it == HT - 1))
                    scorrs = {}
                    for ui, (tt, sc) in enumerate(units):
                        scorrs[(tt, sc)] = psc.tile([128, 512], F32,
                                                    tag="mmc",
                                                    name=f"scorr{ui % 2}")
                        for pi, (lhs, rhs) in enumerate(
                                ((zl8_t, eh8_t), (zh8_t, el8_t))):
                            for k in range(HT // 2):
                                nc.tensor.matmul(
                                    scorrs[(tt, sc)][:],
                                    lhs[:, 2 * k:2 * k + 2,
                                        tt * 128:(tt + 1) * 128],
                                    rhs[:, 2 * k:2 * k + 2,
                                        sc * 512:(sc + 1) * 512],
                                    start=(pi == 0 and k == 0),
                                    stop=(pi == 1 and k == HT // 2 - 1),
                                    perf_mode=DR)
                    for ui, (tt, sc) in enumerate(units):
                        chunk = sc_tiles[tt][:, sc * 512:(sc + 1) * 512]
                        nc.scalar.activation(
                            out=chunk, in_=spss[(tt, sc)][:],
                            func=mybir.ActivationFunctionType.Copy)
                        nc.vector.scalar_tensor_tensor(
                            out=chunk, in0=scorrs[(tt, sc)][:],
                            scalar=1.0 / (SC_ZL * SC_EH),
                            in1=chunk,
                            op0=mybir.AluOpType.mult,
                            op1=mybir.AluOpType.add)
                        if sc == 1:
                            # both chunks of tt merged: softmax over free dim
                            sc_t = sc_tiles[tt]
                            negmax = stp.tile([128, 1], F32, tag="nm")
                            nc.vector.reduce_max(
                                out=negmax[:], in_=sc_t[:],
                                axis=mybir.AxisListType.X, negate=True)
                            p_t = pp.tile([128, S], F16, tag="p")
                            ssum = stp.tile([128, 1], F32, tag="ss")
                            nc.scalar.activation(
                                out=p_t[:], in_=sc_t[:],
                                func=mybir.ActivationFunctionType.Exp,
                                bias=negmax[:], scale=1.0,
                                accum_out=ssum[:])
                            rsum = stp.tile([128, 1], F32, tag="rs", bufs=8)
                            nc.vector.reciprocal(rsum[:], ssum[:])
                            rsums.append(rsum)
                            if NORM == "now":
                                nc.vector.tensor_scalar_mul(
                                    p_t[:], p_t[:], rsum[:])
                            p_tiles.append(p_t)
                            # transpose p -> pT [s, t] via DMA xbar
                            if TR == "dma":
                                for st in range(ST):
                                    nc.sync.dma_start(
                                        pt_t[:, st, tt * 128:(tt + 1) * 128],
                                        p_t[:, st * 128:(st + 1) * 128],
                                        transpose=True)

                if TR == "pe":
                    # st-major quads: 4 transposes land in one psum bank and
                    # evict as a single [128,512] pt slab, so the PE paces at
                    # the transpose rate instead of the eviction round-trip.
                    for st in range(ST):
                        tps = psc.tile([128, TT, 128], F16, tag="tr", bufs=2)
                        for tt in range(TT):
                            nc.tensor.transpose(
                                tps[:, tt, :],
                                p_tiles[tt][:, st * 128:(st + 1) * 128],
                                ident[:])
                        nc.scalar.activation(
                            out=pt_t[:, st, :], in_=tps[:],
                            func=mybir.ActivationFunctionType.Copy)

                # --- cT = enc_nat.T @ pT -> [k, t] f16 (unnormalized) ---
                ct_t = ctp.tile([128, HT, T], F16, tag="ct")
                for kt in range(HT):
                    cps = psmm.tile([128, T], F32, tag="mm")
                    for st in range(ST):
                        nc.tensor.matmul(
                            cps[:],
                            en_t[:, st, kt * 128:(kt + 1) * 128],
                            pt_t[:, st, :],
                            start=(st == 0), stop=(st == ST - 1))
                    nc.scalar.activation(
                        out=ct_t[:, kt, :], in_=cps[:],
                        func=mybir.ActivationFunctionType.Copy)

                # --- out = tanh(cT.T @ WcT * rsum + qT.T @ WqT + b) ---
                for tt in range(TT):
                    for hc in range(2):
                        # q-part first: gives tail cT evictions extra slack
                        oq = psc.tile([128, 512], F32, tag="mmc")
                        for ht in range(HT):
                            nc.tensor.matmul(
                                oq[:],
                                qh_t[:, ht, tt * 128:(tt + 1) * 128],
                                wo_t[:, HT + ht, hc * 512:(hc + 1) * 512],
                                start=(ht == 0), stop=(ht == HT - 1))
                        oc = psmm.tile([128, 512], F32, tag="mm")
                        for kt in range(HT):
                            nc.tensor.matmul(
                                oc[:],
                                ct_t[:, kt, tt * 128:(tt + 1) * 128],
                                wo_t[:, kt, hc * 512:(hc + 1) * 512],
                                start=(kt == 0), stop=(kt == HT - 1))
                        ost = op.tile([128, 512], F32 if has_bias else F16,
                                      tag="os", bufs=2)
                        if NORM == "defer":
                            ocn = scrp.tile([128, 512], F32, tag="comb")
                            nc.vector.tensor_scalar_mul(
                                ocn[:], oc[:], rsums[tt][:])
                            nc.vector.tensor_add(ost[:], ocn[:], oq[:])
                        else:
                            nc.vector.tensor_add(ost[:], oc[:], oq[:])
                        if has_bias:
                            ost16 = op.tile([128, 512], F16, tag="os16")
                            nc.vector.tensor_add(
                                ost[:], ost[:],
                                bias_t[:, hc * 512:(hc + 1) * 512])
                            nc.scalar.activation(
                                out=ost16[:], in_=ost[:],
                                func=mybir.ActivationFunctionType.Tanh)
                            ost = ost16
                        else:
                            nc.scalar.activation(
                                out=ost[:], in_=ost[:],
                                func=mybir.ActivationFunctionType.Tanh)
                        nc.sync.dma_start(
                            out_d[b, tt * 128:(tt + 1) * 128,
                                  hc * 512:(hc + 1) * 512],
                            ost[:])

    nc.compile()
    return nc


def _get_nc(has_bias):
    key = ("nc", has_bias, TR, NORM)
    if key not in _CACHE:
        _CACHE[key] = _build(has_bias)
    return _CACHE[key]


def _split16(x):
    hi = x.astype(np.float16)
    lo = (x - hi.astype(np.float32)).astype(np.float32)
    return hi, lo


def _f8(x, scale):
    return (np.asarray(x, np.float32) * np.float32(scale)).astype(
        ml_dtypes.float8_e4m3)


def kernel(query, encoder_outputs, src_lengths, W_in, W_out, b_out):
    query = np.asarray(query, np.float32)
    enc = np.asarray(encoder_outputs, np.float32)
    W_in = np.asarray(W_in, np.float32)
    W_out = np.asarray(W_out, np.float32)
    b_out = np.asarray(b_out, np.float32)
    has_bias = bool(np.any(b_out))

    # host-side layout prep (transposes + fp16 hi/lo splits)
    qT = np.ascontiguousarray(query.transpose(0, 2, 1))        # [B, H, T]
    qh, ql = _split16(qT)
    encT = np.ascontiguousarray(enc.transpose(1, 2, 0))        # [B, H, S]
    eh, el = _split16(encT)
    en = np.ascontiguousarray(enc.transpose(1, 0, 2)).astype(np.float16)
    whf, wlf = _split16(np.ascontiguousarray(W_in.T))          # [H(h), H(i)]
    wo = np.ascontiguousarray(W_out.T).astype(np.float16)      # [2H, H]

    common = {"wh": whf, "wo": wo,
              "wh8": _f8(whf.astype(np.float32), SC_WH),
              "wl8": _f8(wlf, SC_WL)}
    if has_bias:
        common["bias"] = np.ascontiguousarray(
            np.broadcast_to(b_out[None, :], (128, H)), np.float32)

    in_maps = []
    for c in range(NCORES):
        sl = slice(c * BL, (c + 1) * BL)
        m = {
            "qh": np.ascontiguousarray(qh[sl]),
            "eh": np.ascontiguousarray(eh[sl]),
            "en": np.ascontiguousarray(en[sl]),
            "qh8": _f8(qh[sl].astype(np.float32), SC_QH),
            "ql8": _f8(ql[sl], SC_QL),
            "eh8": _f8(eh[sl].astype(np.float32), SC_EH),
            "el8": _f8(el[sl], SC_EL),
            **common,
        }
        in_maps.append(m)

    nc = _get_nc(has_bias)
    trace = bool(int(os.environ.get("KERNEL_TRACE", "0")))
    res = run_bass_kernel_spmd(nc, in_maps, core_ids=list(range(NCORES)),
                               trace=trace)
    if trace:
        _CACHE["last_exec_time_ns"] = res.exec_time_ns
        _CACHE["last_results"] = res
    out = np.concatenate([r["out"] for r in res.results], axis=0)
    return out.astype(np.float32)
